# revision 60
# baseline (speedup 1.0000x reference)
"""Trainium2 Bass kernel for nn_DistanceBias (gnn_message_passing).

Math (derived from the reference):
  out[b,h,r,c] = ef(dist(r,c))[h] + vec(pos_c - pos_r)[h]   if r < L or c < L
               = 0                                           otherwise
with L = N - protein_length = 256 ligand nodes,
  dist(r,c) = 1/(|pos_r - pos_c|^2 + 1)  in (0, 1],
  ef(d)  = gelu(G(d) @ ow1 + ob1) @ ow2 + ob2,   G_k(d) = gaussian features
  vec(u) = gelu(u @ vw1 + vb1) @ vw2 + vb2.

Key structure exploited:
  * With constant mul_w/bias_w tables the whole edge-feature path ef(d) is a
    smooth scalar function of d in (0,1].  It is fit ONCE on the host as a
    degree-24 Chebyshev series; the device evaluates the basis per pair with a
    cheap fp16 product ladder (V_b = 2*T_b obeys V_{m+n} = V_m*V_n - V_|m-n|)
    and contracts basis x coefficients in the SAME PSUM matmul accumulation as
    the vector-path projection.  This removes the exp activation, the 128x128
    MLP matmuls and their gelu - the activation engine only runs the
    vector-path gelu (one column per pair).
  * dist is symmetric and (vb1 == 0) gelu(-u) = gelu(u) - u, so each unordered
    pair is computed ONCE; mirrors are reconstructed on the host from the
    rank-3 correction sv = pos @ (vw1 @ vw2).
  * The vector-path subtraction u = tvJ[:,j] - tvI[:,i] is built on DVE in the
    2x fp16 mode: the host sends tvI with every column DOUBLED so that all
    three operands end in a packed [1,2] access-pattern dim.
  * The Chebyshev basis is built in pair-tile layout [i, j] and transposed to
    matmul layout [basis, pair] with two large DMAs through a DRAM scratch
    (arbitrary DRAM access patterns make the reshape free).
  * The protein x protein quadrant (56% of output) is exactly zero and never
    touched on device.  Device outputs are fp16 blocked slabs; the host
    reshapes, adds the shared output bias (ob2 + vb2 + c0) and applies
    mirrors.

Work is split into an identical 5-job program per core (3 full 128x128 mirror
blocks + one 64x64 mirror quarter + one 64x128 ordered diag half = 61440
computed pairs per core, exactly 1/8 of the total).
"""

import os
import sys

import numpy as np

# ---------------------------------------------------------------------------
# problem constants (hardcoded per task instructions)
N = 1024
PLEN = 768
LIG = 256
K = 128
H = 32
B = 2
DCHEB = 24  # Chebyshev degree: rows V_1..V_D on device, c_0 folded into bias
A_CONST = (2.0 * 3.14159) ** 0.5  # matches reference PI

F32 = np.float32
F16 = np.float16


def _ensure_concourse():
    try:
        import concourse  # noqa: F401
        return
    except ImportError:
        pass
    for p in ("/opt/trn_rl_repo", "/root/.axon_site/_ro/trn_rl_repo"):
        if os.path.isdir(p) and p not in sys.path:
            sys.path.insert(0, p)
    import concourse  # noqa: F401


# ---------------------------------------------------------------------------
# job tables


def make_jobs():
    """Per-core job lists. Job = (batch, i0, I, j0, J, mirror).

    Fixed per-core structure (same shapes on every core so that a single
    SPMD program serves all 8 cores):
      jobs[0..2] : full 128x128 mirror blocks (LP region)
      jobs[3]    : 64x64 mirror quarter (LL off-diagonal block)
      jobs[4]    : 64x128 ordered diag half (LL diagonal blocks)
    """
    hd = [(0, 64, 0, 128), (64, 64, 0, 128), (128, 64, 128, 128), (192, 64, 128, 128)]
    qq = [(0, 64, 128, 64), (0, 64, 192, 64), (64, 64, 128, 64), (64, 64, 192, 64)]
    lp = []
    for b in range(B):
        for t in range(6):
            for jj in range(2):
                lp.append((b, 256 + 128 * t, 128, 128 * jj, 128, True))
    cores = []
    for c in range(8):
        b = c // 4
        jobs = list(lp[3 * c : 3 * c + 3])
        i0, I, j0, J = qq[c % 4]
        jobs.append((b, i0, I, j0, J, True))
        i0, I, j0, J = hd[c % 4]
        jobs.append((b, i0, I, j0, J, False))
        cores.append(jobs)
    return cores


# job slot shapes shared by the program on every core: (I, J, mirror)
JOB_SLOTS = [(128, 128, True), (128, 128, True), (128, 128, True),
             (64, 64, True), (64, 128, False)]


BASE_DEG = 6  # exact V-recurrence up to here; higher rows are pure products


def _vchain(b):
    """Operands (m, n, k) with V_b = V_m*V_n - V_k, all indices < b."""
    if b % 2 == 0:
        return (b // 2 + 1, b // 2 - 1, 2)
    return ((b + 1) // 2, (b - 1) // 2, 1)


# ---------------------------------------------------------------------------
# numpy fallback (exact reference math) for input shapes/values outside the
# fast path.  kernel.py must be self-contained, so this re-implements the
# reference directly.


def _np_gelu(x):
    z = np.asarray(x, np.float64) / np.sqrt(2.0)
    try:
        from scipy.special import erf
        e = erf(z)
    except ImportError:
        import math
        e = np.vectorize(math.erf)(z)
    return np.asarray(x, np.float64) * (0.5 * (1.0 + e))


def _np_nonlinear(x, w1, b1, w2, b2):
    return (_np_gelu(np.asarray(x, F32) @ w1 + b1) @ w2 + b2).astype(F32)


def _np_gaussian(dist, etype, mul_w, bias_w, means, stds):
    mul = mul_w[etype]
    bias = bias_w[etype]
    x = mul * dist[..., None] + bias
    x = x - means
    std = np.abs(stds) + 1e-5
    return (np.exp(-0.5 * (x / std) ** 2) / (A_CONST * std)).astype(F32)


def _numpy_reference(pos, edge_types, protein_length, means, stds, mul_w, bias_w,
                     ow1, ob1, ow2, ob2, vw1, vb1, vw2, vb2):
    pos = np.asarray(pos, F32)
    Bv, Nv, _ = pos.shape
    P = int(protein_length)
    L = Nv - P
    Hv = ow2.shape[1]
    lig = pos[:, :L]
    prot = pos[:, L:]
    dlm_ll = lig[:, None, :, :] - lig[:, :, None, :]
    dlm_lp = lig[:, None, :, :] - prot[:, :, None, :]
    dist_ll = 1.0 / ((dlm_ll ** 2).sum(-1) + 1.0)
    dist_lp = 1.0 / ((dlm_lp ** 2).sum(-1) + 1.0)
    dlm_ll_h = _np_nonlinear(dlm_ll, vw1, vb1, vw2, vb2)
    dlm_pl_h = _np_nonlinear(-dlm_lp, vw1, vb1, vw2, vb2)
    dlm_lp_h = _np_nonlinear(dlm_lp, vw1, vb1, vw2, vb2)
    g_ll = _np_gaussian(dist_ll, edge_types[:, :L, :L], mul_w, bias_w, means, stds)
    ef_ll = _np_nonlinear(g_ll, ow1, ob1, ow2, ob2)
    g_lp = _np_gaussian(dist_lp, edge_types[:, L:, :L], mul_w, bias_w, means, stds)
    ef_lp = _np_nonlinear(g_lp, ow1, ob1, ow2, ob2)
    ef = np.zeros((Bv, Nv, Nv, Hv), F32)
    ef[:, :L, :L, :] = ef_ll + dlm_ll_h
    ef[:, L:, :L, :] = ef_lp + dlm_lp_h
    ef[:, :L, L:, :] = np.swapaxes(ef_lp + dlm_pl_h, 1, 2)
    return np.transpose(ef, (0, 3, 1, 2)).copy()


# ---------------------------------------------------------------------------
# host-side Chebyshev fit of the edge-feature path


def _vladder_fp16(d32):
    """Simulate the device fp16 basis ladder exactly: V-recurrence up to
    BASE_DEG, then products V_b = V_BASE * V_{b-BASE} for higher degrees."""
    D = DCHEB
    V = [None] * (D + 1)
    V[1] = (4.0 * d32.astype(F32) - 2.0).astype(F16)
    V[2] = ((V[1] * V[1]).astype(F16).astype(F32) - 2.0).astype(F16)
    for b in range(3, BASE_DEG + 1):
        m, n, k = _vchain(b)
        V[b] = ((V[m] * V[n]).astype(F16) - V[k]).astype(F16)
    for b in range(BASE_DEG + 1, D + 1):
        V[b] = (V[BASE_DEG] * V[b - BASE_DEG]).astype(F16)
    return np.stack(V[1:], 0)  # [D, M]


def _fit_ef_cheb(means, stds, ow1, ob1, ow2):
    """Least-squares fit of ef(d) (without ob2) on d in (0,1] against the
    exact fp16 device basis.  Returns co [D+1, 32] (row 0 = constant)."""
    M = 16384
    dgrid = (np.arange(M, dtype=np.float64) + 0.5) / M
    s = np.abs(stds) + 1e-5
    xg = (dgrid[:, None] - means) / s
    G = np.exp(-0.5 * xg * xg) / (A_CONST * s)
    f = _np_gelu(G @ ow1 + ob1) @ ow2       # [M, 32] float64
    Vd = _vladder_fp16(dgrid).astype(np.float64)          # [D, M]
    Bm = np.concatenate([np.ones((1, M)), Vd], axis=0).T  # [M, D+1]
    co, *_ = np.linalg.lstsq(Bm, f, rcond=None)           # [D+1, 32]
    return co, dgrid, f


def _fit_error(co, dgrid, f):
    cm = co[1:].astype(F16).astype(F32)     # [D, 32] as sent to device
    Vd = _vladder_fp16(dgrid)
    est = Vd.astype(F32).T @ cm + co[0][None, :].astype(F32)
    return float(np.abs(est - f).max())


# ---------------------------------------------------------------------------
# device program


_PROGRAM_CACHE = {}


def _build_program():
    """Build the SPMD Bass program (identical for all 8 cores)."""
    _ensure_concourse()
    import contextlib

    import concourse.bass as bass  # noqa: F401
    import concourse.tile as tile
    from concourse import bacc, mybir
    from concourse.tile import add_dep_helper

    dt = mybir.dt
    AF = mybir.ActivationFunctionType
    ALU = mybir.AluOpType

    D = DCHEB
    nc = bacc.Bacc("TRN2", target_bir_lowering=False, debug=False)

    def din(name, shape, dd=None):
        return nc.dram_tensor(name, list(shape), dd or dt.float32,
                              kind="ExternalInput").ap()

    CM = din("CM", (K, H), dt.float16)   # rows 0..D-1 = cheb c_b / 2
    V2 = din("V2", (K, H), dt.float16)   # vw2
    jin, jout, jscr = [], [], []
    for jidx, (I, J, mirror) in enumerate(JOB_SLOTS):
        NP = I * J
        jin.append({
            "fg": din(f"fg{jidx}", (5, I + J)),                    # gl | gr
            "tv": din(f"tv{jidx}", (K, 2 * I + J), dt.float16),    # tvI2 | tvJ
        })
        jout.append({"od": nc.dram_tensor(
            f"od{jidx}", [K, NP // 4], dt.float16, kind="ExternalOutput").ap()})
        jscr.append(nc.dram_tensor(
            f"bs{jidx}", [D, NP], dt.float16, kind="Internal").ap())

    def raw(inst):
        return inst.ins if hasattr(inst, "ins") else inst

    with tile.TileContext(nc) as tc:
        stack = contextlib.ExitStack()
        consts = stack.enter_context(tc.tile_pool(name="consts", bufs=1))
        vpool = stack.enter_context(tc.tile_pool(name="vpool", bufs=1))
        bpool = stack.enter_context(tc.tile_pool(name="bpool", bufs=2))
        bpoolS = stack.enter_context(tc.tile_pool(name="bpoolS", bufs=1))
        upool = stack.enter_context(tc.tile_pool(name="upool", bufs=7))
        hvpool = stack.enter_context(tc.tile_pool(name="hvpool", bufs=8))
        stpool = stack.enter_context(tc.tile_pool(name="stpool", bufs=4))
        psR = stack.enter_context(tc.tile_pool(name="psR", bufs=1, space="PSUM"))
        psO = stack.enter_context(tc.tile_pool(name="psO", bufs=6, space="PSUM"))

        # job0 inputs land first so its pipeline starts immediately
        sbj = [None] * len(JOB_SLOTS)
        for jidx in [0]:
            t = {}
            for kind in ("tv", "fg"):
                shp = list(jin[jidx][kind].shape)
                dd = dt.float16 if kind == "tv" else dt.float32
                t[kind] = consts.tile(shp, dd, name=f"jc_{kind}{jidx}")
                nc.sync.dma_start(out=t[kind][:, :], in_=jin[jidx][kind])
            sbj[jidx] = t
        CM_s = consts.tile([K, H], dt.float16, name="cm")
        nc.sync.dma_start(out=CM_s[:, :], in_=CM)
        V2_s = consts.tile([K, H], dt.float16, name="v2")
        nc.sync.dma_start(out=V2_s[:, :], in_=V2)
        for jidx in range(1, len(JOB_SLOTS)):
            t = {}
            for kind in ("fg", "tv"):
                shp = list(jin[jidx][kind].shape)
                dd = dt.float16 if kind == "tv" else dt.float32
                t[kind] = consts.tile(shp, dd, name=f"jc_{kind}{jidx}")
                nc.sync.dma_start(out=t[kind][:, :], in_=jin[jidx][kind])
            sbj[jidx] = t

        # ---- d = 1/(r^2+1) per job into one [128, 640] tile ---------------
        D_all = vpool.tile([K, 128 * len(JOB_SLOTS)], dt.float32, name="D_all")
        nc.gpsimd.memset(D_all[:, :], 0.0)
        # warmup activation: hoists the auto-inserted Gelu table load (1.3us)
        # into the head idle instead of delaying the first real gelu
        warm = vpool.tile([K, 1], dt.float16, name="warm")
        nc.scalar.activation(warm[:, :], D_all[:, 0:1], AF.Gelu, bias=0.0)

        def emit_recip(jidx):
            I, J, _ = JOB_SLOTS[jidx]
            joff = 128 * jidx
            fg = sbj[jidx]["fg"]
            pR = psR.tile([128, 128], dt.float32, tag="r", name="pR")
            nc.tensor.matmul(pR[:I, :J], fg[:, 0:I], fg[:, I:I + J],
                             start=True, stop=True)
            nc.vector.reciprocal(D_all[:I, joff:joff + J], pR[:I, :J])

        # ---- fp16 basis ladder --------------------------------------------
        # Exact V-recurrence (V_b = 2*T_b) up to BASE_DEG, then pure products
        # V_b = V_BASE * V_{b-BASE}: 29 DVE ops instead of 44, and O(log)
        # dependency depth.  Split in two column ranges: job0's 128 cols
        # first (unblocks the first basis DMA early), then the rest.
        NC = 128 * len(JOB_SLOTS)
        V_all = vpool.tile([K, D * NC], dt.float16, name="V_all")

        def ladder(c0, c1):
            w = c1 - c0

            def V(b):
                return V_all[:, (b - 1) * NC + c0:(b - 1) * NC + c1]

            nc.vector.tensor_scalar(V(1), D_all[:, c0:c1], 4.0, -2.0,
                                    ALU.mult, ALU.add)
            mt = vpool.tile([K, w], dt.float16, name=f"lm0_{c0}",
                            tag=f"lm0_{w}")
            nc.vector.tensor_mul(mt[:, :w], V(1), V(1))
            nc.vector.tensor_scalar(V(2), mt[:, :w], -2.0, None, ALU.add)
            for b in range(3, BASE_DEG + 1):
                m, n, k = _vchain(b)
                mm = vpool.tile([K, w], dt.float16, name=f"lm{b}_{c0}",
                                tag=f"lm{b % 2}_{w}")
                nc.vector.tensor_mul(mm[:, :w], V(m), V(n))
                nc.vector.tensor_sub(V(b), mm[:, :w], V(k))
            for b in range(BASE_DEG + 1, D + 1):
                nc.vector.tensor_mul(V(b), V(BASE_DEG), V(b - BASE_DEG))

        d1_insts = {}

        def emit_d1(jidx, i0=0, i1=None):
            I, J, _ = JOB_SLOTS[jidx]
            if i1 is None:
                i1 = I
            joff = 128 * jidx
            vsl = V_all[:, :].rearrange("p (b c) -> p b c", b=D,
                                        c=NC)[i0:i1, :, joff:joff + J]
            d1_insts[(jidx, i0)] = nc.sync.dma_start(
                out=jscr[jidx][:, i0 * J:i1 * J].rearrange(
                    "b (i j) -> i b j", i=i1 - i0, j=J),
                in_=vsl)

        Bts = {}

        def emit_d2(jidx, i0=0, i1=None):
            I, J, _ = JOB_SLOTS[jidx]
            NP = I * J
            if i1 is None:
                i1 = I
            if jidx in Bts:
                Bt = Bts[jidx]
            elif NP == 16384:
                # jobs 0..2 rotate two big buffers; jobs 3/4 get their own
                # smaller tiles so their loads never wait on buffer reuse
                Bt = bpool.tile([128, NP], dt.float16, tag="B",
                                name=f"Bt{jidx}")
            else:
                Bt = bpoolS.tile([128, NP], dt.float16, tag=f"Bs{jidx}",
                                 name=f"Bt{jidx}")
            d2 = nc.sync.dma_start(out=Bt[0:D, i0 * J:i1 * J],
                                   in_=jscr[jidx][:, i0 * J:i1 * J])
            add_dep_helper(raw(d2), raw(d1_insts[(jidx, i0)]), sync=True,
                           reason="bscratch RAW")
            Bts[jidx] = Bt

        def emit_ut(jidx, h, Ut, off, eng=None, sub=None):
            I, J, _ = JOB_SLOTS[jidx]
            iin = 2048 // J
            iw0 = h * iin
            w = 2048
            if sub is not None:
                iin //= 2
                iw0 += sub * iin
                off += sub * 1024
                w = 1024
            tv = sbj[jidx]["tv"]
            (eng or nc.vector).tensor_tensor(
                Ut[:, off:off + w].rearrange(
                    "p (ii jj j2) -> p ii jj j2", ii=iin, jj=J // 2, j2=2),
                tv[:, 2 * I:2 * I + J][:, None, :].broadcast_to(
                    [128, iin, J]).rearrange(
                    "p ii (jj j2) -> p ii jj j2", j2=2),
                tv[:, 2 * iw0:2 * (iw0 + iin)].rearrange(
                    "p (ii j2) -> p ii j2", j2=2)[:, :, None, :]
                .broadcast_to([128, iin, J // 2, 2]),
                ALU.subtract)

        # ---- schedule -----------------------------------------------------
        # 30 uniform 2048-pair half-group units.  Pool builds a unit's Ut in
        # 4.2us, DVE in 1.1us, ACT consumes one every 1.9us, so Pool covers
        # roughly every other slot while DVE runs the basis ladder pieces.
        # Drains lag their slot by 4 so the in-order DVE queue never parks
        # on an unfinished PSUM tile; output DMAs pair two consecutive
        # halves and follow the odd drain.
        SCHED = []
        for jidx in (0, 1, 2, 4, 3):
            I, J, _ = JOB_SLOTS[jidx]
            SCHED += [(jidx, h) for h in range(I * J // 2048)]
        POOL_SLOTS = {2, 4, 6, 9, 12, 14, 17, 19, 21, 23, 24, 26}

        pOs, sts = {}, {}

        def emit_proj(item, hvt, off):
            jidx, h = item
            Bt = Bts[jidx]
            pO = psO.tile([128, 512], dt.float32, tag="o", name="pO")
            for c in range(16):
                ch0 = h * 2048 + c * 128
                cs = 32 * c
                nc.tensor.matmul(pO[:, cs:cs + 32], Bt[0:D, ch0:ch0 + 128],
                                 CM_s[0:D, :], start=True, stop=False)
                nc.tensor.matmul(pO[:, cs:cs + 32],
                                 hvt[:, off + c * 128:off + (c + 1) * 128],
                                 V2_s[:, :], start=False, stop=True)
            pOs[item] = pO

        def emit_drain(item, on_act=False, split_out=False):
            jidx, h = item
            if h % 2 == 0:
                sts[jidx] = stpool.tile([128, 1024], dt.float16, tag="st",
                                        name="st")
            st = sts[jidx]
            sl = (h % 2) * 512
            if on_act:
                nc.scalar.activation(st[:, sl:sl + 512], pOs.pop(item)[:, :],
                                     AF.Copy)
            else:
                nc.vector.tensor_copy(st[:, sl:sl + 512], pOs.pop(item)[:, :])
            if split_out:
                # final pair: per-half outputs so the very last DMA is small
                nc.sync.dma_start(out=jout[jidx]["od"][:, h * 512:(h + 1) * 512],
                                  in_=st[:, sl:sl + 512])
            elif h % 2 == 1:
                nc.sync.dma_start(
                    out=jout[jidx]["od"][:, (h - 1) * 512:(h + 1) * 512],
                    in_=st[:, :])

        prehooks = {
            # between slot 0's gelu and its projection: job0 basis pipeline,
            # first half (pairs 0..8191) so the projections start early
            0: lambda: (emit_recip(0), ladder(0, 128),
                        emit_d1(0, 0, 64), emit_d2(0, 0, 64)),
        }
        hooks = {
            1: lambda: (emit_d1(0, 64, 128), emit_d2(0, 64, 128)),
            2: lambda: [emit_recip(j) for j in range(1, 5)],
            3: lambda: (ladder(128, 384), emit_d1(1), emit_d1(2), emit_d2(1)),
            8: lambda: (ladder(384, NC), emit_d1(4), emit_d1(3), emit_d2(2)),
            10: lambda: emit_d2(4),
            12: lambda: emit_d2(3),
        }
        for k, item in enumerate(SCHED):
            if k >= 3:
                emit_drain(SCHED[k - 3])
            eng = nc.gpsimd if k in POOL_SLOTS else nc.vector
            Ut = upool.tile([128, 2048], dt.float16, tag="u", name="Ut")
            hvt = hvpool.tile([128, 2048], dt.float16, tag="hv", name="hvt")
            emit_ut(*item, Ut=Ut, off=0, eng=eng)
            nc.scalar.activation(hvt[:, :], Ut[:, :], AF.Gelu, bias=0.0)
            if k in prehooks:
                prehooks[k]()
            emit_proj(item, hvt, 0)
            if k in hooks:
                hooks[k]()
        n = len(SCHED)
        for k in range(n, n + 3):
            # trailing drains stay off ACT: the gelu stream IS the makespan
            emit_drain(SCHED[k - 3], on_act=(k == n + 2))

        stack.close()

    nc.compile()
    return nc, {}


def _get_program():
    if "prog" not in _PROGRAM_CACHE:
        _PROGRAM_CACHE["prog"] = _build_program()
    return _PROGRAM_CACHE["prog"]


# ---------------------------------------------------------------------------
# host side


def _prep_core_inputs(core_jobs, pos, tvT_all, n2_all, consts):
    """Build the input map for one core."""
    m = dict(consts)
    for jidx, (b, i0, I, j0, J, mirror) in enumerate(core_jobs):
        p = pos[b]
        n2 = n2_all[b]
        tvT = tvT_all[b]
        fg = np.empty((5, I + J), F32)
        fg[0:3, :I] = -2.0 * p[i0:i0 + I].T
        fg[3, :I] = n2[i0:i0 + I]
        fg[4, :I] = 1.0
        fg[0:3, I:] = p[j0:j0 + J].T
        fg[3, I:] = 1.0
        fg[4, I:] = n2[j0:j0 + J] + 1.0
        tv = np.empty((K, 2 * I + J), F16)
        tv[:, 0:2 * I] = np.repeat(tvT[:, i0:i0 + I], 2, axis=1)
        tv[:, 2 * I:] = tvT[:, j0:j0 + J]
        m[f"fg{jidx}"] = np.ascontiguousarray(fg)
        m[f"tv{jidx}"] = np.ascontiguousarray(tv)
    return m


_RUNNER_CACHE = {}


def _get_runner(nc):
    """Compile (once) a jitted shard_map over the 8 cores with donated,
    device-side-created zero output buffers."""
    if "r" in _RUNNER_CACHE:
        return _RUNNER_CACHE["r"]
    _ensure_concourse()
    import jax
    import jax.numpy as jnp
    from jax.sharding import Mesh, NamedSharding, PartitionSpec
    from jax.experimental.shard_map import shard_map
    from concourse import mybir
    from concourse.bass2jax import (_bass_exec_p, install_neuronx_cc_hook,
                                    partition_id_tensor)

    install_neuronx_cc_hook()

    in_names, out_names, out_avals = [], [], []
    partition_name = (nc.partition_id_tensor.name
                      if nc.partition_id_tensor else None)
    for alloc in nc.m.functions[0].allocations:
        if not isinstance(alloc, mybir.MemoryLocationSet):
            continue
        name = alloc.memorylocations[0].name
        if alloc.kind == "ExternalInput":
            if name != partition_name:
                in_names.append(name)
        elif alloc.kind == "ExternalOutput":
            out_names.append(name)
            out_avals.append(jax.core.ShapedArray(
                tuple(alloc.tensor_shape), mybir.dt.np(alloc.dtype)))
    n_params = len(in_names)
    n_outs = len(out_avals)
    all_in_names = list(in_names) + list(out_names)
    if partition_name is not None:
        all_in_names.append(partition_name)

    def _body(*args):
        operands = list(args)
        if partition_name is not None:
            operands.append(partition_id_tensor())
        outs = _bass_exec_p.bind(
            *operands, out_avals=tuple(out_avals),
            in_names=tuple(all_in_names), out_names=tuple(out_names),
            lowering_input_output_aliases=(), sim_require_finite=True,
            sim_require_nnan=True, nc=nc)
        return tuple(outs)

    devices = jax.devices()[:8]
    mesh = Mesh(np.asarray(devices), ("core",))
    in_specs = (PartitionSpec("core"),) * (n_params + n_outs)
    out_specs = (PartitionSpec("core"),) * n_outs
    donate = tuple(range(n_params, n_params + n_outs))
    sharded = jax.jit(
        shard_map(_body, mesh=mesh, in_specs=in_specs, out_specs=out_specs,
                  check_rep=False),
        donate_argnums=donate, keep_unused=True)

    zshapes = [(8 * a.shape[0], *a.shape[1:]) for a in out_avals]
    zdtypes = [a.dtype for a in out_avals]
    mk = jax.jit(lambda: tuple(jnp.zeros(s, d)
                               for s, d in zip(zshapes, zdtypes)),
                 out_shardings=tuple(
                     NamedSharding(mesh, PartitionSpec("core"))
                     for _ in range(n_outs)))

    _RUNNER_CACHE["r"] = (sharded, mk, in_names, out_names, out_avals)
    return _RUNNER_CACHE["r"]


def _run_on_device(nc, in_maps):
    import jax

    sharded, mk, in_names, out_names, out_avals = _get_runner(nc)
    per_core = [[np.asarray(m[name]) for name in in_names] for m in in_maps]
    concat_in = [np.concatenate([per_core[c][i] for c in range(8)], axis=0)
                 for i in range(len(in_names))]
    out_arrs = jax.block_until_ready(sharded(*concat_in, *mk()))
    results = []
    for c in range(8):
        results.append({
            name: np.asarray(out_arrs[i]).reshape(8, *out_avals[i].shape)[c]
            for i, name in enumerate(out_names)})
    return results


def _decode_direct(arr, I, J):
    """[128, NP/4] fp16 pair-chunked slabs -> [H, I, J] fp32.

    arr[p, chunk*32 + h] holds pair n = chunk*128 + p, n = i*J + j.
    """
    NP = I * J
    nch = NP // 128
    a = arr.astype(F32).reshape(128, nch, 32)     # [p, chunk, h]
    a = a.transpose(1, 0, 2).reshape(NP, 32)      # pair-major [n, h]
    return a.reshape(I, J, 32).transpose(2, 0, 1)


def kernel(**inputs):
    pos = np.ascontiguousarray(np.asarray(inputs["pos"], F32))
    protein_length = int(np.asarray(inputs["protein_length"]))
    means = np.asarray(inputs["means"], np.float64)
    stds = np.asarray(inputs["stds"], np.float64)
    mul_w = np.asarray(inputs["mul_w"], F32)
    bias_w = np.asarray(inputs["bias_w"], F32)
    ow1 = np.asarray(inputs["ow1"], F32)
    ob1 = np.asarray(inputs["ob1"], F32)
    ow2 = np.asarray(inputs["ow2"], F32)
    ob2 = np.asarray(inputs["ob2"], F32)
    vw1 = np.asarray(inputs["vw1"], F32)
    vb1 = np.asarray(inputs["vb1"], F32)
    vw2 = np.asarray(inputs["vw2"], F32)
    vb2 = np.asarray(inputs["vb2"], F32)

    def _fallback():
        return _numpy_reference(pos, np.asarray(inputs["edge_types"]),
                                protein_length, means.astype(F32),
                                np.asarray(stds, F32), mul_w, bias_w, ow1, ob1,
                                ow2, ob2, vw1, vb1, vw2, vb2)

    fast_ok = (
        pos.shape == (B, N, 3)
        and protein_length == PLEN
        and means.shape == (K,)
        and ow1.shape == (K, K) and ow2.shape == (K, H)
        and vw1.shape == (3, K) and vw2.shape == (K, H)
        and np.all(mul_w == mul_w.reshape(-1)[0])
        and np.all(bias_w == bias_w.reshape(-1)[0])
        and np.all(vb1 == 0.0)
        and float(mul_w.reshape(-1)[0]) == 1.0
        and float(bias_w.reshape(-1)[0]) == 0.0
    )
    if not fast_ok:
        return _fallback()

    # host Chebyshev fit of the edge-feature path, with device-exact check
    co, dgrid, fref = _fit_ef_cheb(means, stds,
                                   ow1.astype(np.float64),
                                   ob1.astype(np.float64),
                                   ow2.astype(np.float64))
    if _fit_error(co, dgrid, fref) > 0.012:
        return _fallback()

    consts = {
        "CM": np.ascontiguousarray(co[1:].astype(F16)),             # [D, 32]
        "V2": np.ascontiguousarray(vw2.astype(F16)),
    }
    consts["CM"] = np.concatenate(
        [consts["CM"], np.zeros((K - DCHEB, H), F16)], axis=0)

    n2_all = (pos.astype(np.float64) ** 2).sum(-1).astype(F32)   # [B, N]
    tvT_all = np.stack([(pos[b] @ vw1).T for b in range(B)], 0).astype(F16)
    w3 = (vw1.astype(np.float64) @ vw2.astype(np.float64))       # [3, 32]
    sv_all = np.stack([(pos[b].astype(np.float64) @ w3).T.astype(F32)
                       for b in range(B)], 0)                    # [B, 32, N]
    outb = (ob2 + vb2 + co[0].astype(F32)).astype(F32)           # [32]

    cores = make_jobs()
    in_maps = [_prep_core_inputs(cores[c], pos, tvT_all, n2_all, consts)
               for c in range(8)]

    try:
        nc, meta = _get_program()
        try:
            results = _run_on_device(nc, in_maps)
        except Exception:
            _ensure_concourse()
            from concourse import bass_utils
            res = bass_utils.run_bass_kernel_spmd(nc, in_maps,
                                                  core_ids=list(range(8)))
            results = res.results
    except Exception:
        # No usable device path in this environment: fall back to the exact
        # host implementation so kernel() always returns a correct result.
        return _fallback()

    out = np.zeros((B, H, N, N), F32)
    bias3 = outb[:, None, None]
    for c in range(8):
        for jidx, (b, i0, I, j0, J, mirror) in enumerate(cores[c]):
            od = _decode_direct(results[c][f"od{jidx}"], I, J)
            out[b, :, i0:i0 + I, j0:j0 + J] = od + bias3
            if mirror:
                # mirror tile: gelu(-u) = gelu(u) - u gives
                # om[h,j,i] = od[h,i,j] - sv[h,j] + sv[h,i]
                sv = sv_all[b]
                out[b, :, j0:j0 + J, i0:i0 + I] = (
                    od.transpose(0, 2, 1) + bias3
                    - sv[:, j0:j0 + J, None] + sv[:, None, i0:i0 + I])
    return out


if __name__ == "__main__":
    nc, meta = _get_program()
    print("program built ok")


# revision 61
# speedup vs baseline: 1.0158x; 1.0158x over previous
"""Trainium2 Bass kernel for nn_DistanceBias (gnn_message_passing).

Math (derived from the reference):
  out[b,h,r,c] = ef(dist(r,c))[h] + vec(pos_c - pos_r)[h]   if r < L or c < L
               = 0                                           otherwise
with L = N - protein_length = 256 ligand nodes,
  dist(r,c) = 1/(|pos_r - pos_c|^2 + 1)  in (0, 1],
  ef(d)  = gelu(G(d) @ ow1 + ob1) @ ow2 + ob2,   G_k(d) = gaussian features
  vec(u) = gelu(u @ vw1 + vb1) @ vw2 + vb2.

Key structure exploited:
  * With constant mul_w/bias_w tables the whole edge-feature path ef(d) is a
    smooth scalar function of d in (0,1].  It is fit ONCE on the host as a
    degree-24 Chebyshev series; the device evaluates the basis per pair with a
    cheap fp16 product ladder (V_b = 2*T_b obeys V_{m+n} = V_m*V_n - V_|m-n|)
    and contracts basis x coefficients in the SAME PSUM matmul accumulation as
    the vector-path projection.  This removes the exp activation, the 128x128
    MLP matmuls and their gelu - the activation engine only runs the
    vector-path gelu (one column per pair).
  * dist is symmetric and (vb1 == 0) gelu(-u) = gelu(u) - u, so each unordered
    pair is computed ONCE; mirrors are reconstructed on the host from the
    rank-3 correction sv = pos @ (vw1 @ vw2).
  * The vector-path subtraction u = tvJ[:,j] - tvI[:,i] is built on DVE in the
    2x fp16 mode: the host sends tvI with every column DOUBLED so that all
    three operands end in a packed [1,2] access-pattern dim.
  * The Chebyshev basis is built in pair-tile layout [i, j] and transposed to
    matmul layout [basis, pair] with two large DMAs through a DRAM scratch
    (arbitrary DRAM access patterns make the reshape free).
  * The protein x protein quadrant (56% of output) is exactly zero and never
    touched on device.  Device outputs are fp16 blocked slabs; the host
    reshapes, adds the shared output bias (ob2 + vb2 + c0) and applies
    mirrors.

Work is split into an identical 5-job program per core (3 full 128x128 mirror
blocks + one 64x64 mirror quarter + one 64x128 ordered diag half = 61440
computed pairs per core, exactly 1/8 of the total).
"""

import os
import sys

import numpy as np

# ---------------------------------------------------------------------------
# problem constants (hardcoded per task instructions)
N = 1024
PLEN = 768
LIG = 256
K = 128
H = 32
B = 2
DCHEB = 20  # Chebyshev degree: rows V_1..V_D on device, c_0 folded into bias
A_CONST = (2.0 * 3.14159) ** 0.5  # matches reference PI

F32 = np.float32
F16 = np.float16


def _ensure_concourse():
    try:
        import concourse  # noqa: F401
        return
    except ImportError:
        pass
    for p in ("/opt/trn_rl_repo", "/root/.axon_site/_ro/trn_rl_repo"):
        if os.path.isdir(p) and p not in sys.path:
            sys.path.insert(0, p)
    import concourse  # noqa: F401


# ---------------------------------------------------------------------------
# job tables


def make_jobs():
    """Per-core job lists. Job = (batch, i0, I, j0, J, mirror).

    Fixed per-core structure (same shapes on every core so that a single
    SPMD program serves all 8 cores):
      jobs[0..2] : full 128x128 mirror blocks (LP region)
      jobs[3]    : 64x64 mirror quarter (LL off-diagonal block)
      jobs[4]    : 64x128 ordered diag half (LL diagonal blocks)
    """
    hd = [(0, 64, 0, 128), (64, 64, 0, 128), (128, 64, 128, 128), (192, 64, 128, 128)]
    qq = [(0, 64, 128, 64), (0, 64, 192, 64), (64, 64, 128, 64), (64, 64, 192, 64)]
    lp = []
    for b in range(B):
        for t in range(6):
            for jj in range(2):
                lp.append((b, 256 + 128 * t, 128, 128 * jj, 128, True))
    cores = []
    for c in range(8):
        b = c // 4
        jobs = list(lp[3 * c : 3 * c + 3])
        i0, I, j0, J = qq[c % 4]
        jobs.append((b, i0, I, j0, J, True))
        i0, I, j0, J = hd[c % 4]
        jobs.append((b, i0, I, j0, J, False))
        cores.append(jobs)
    return cores


# job slot shapes shared by the program on every core: (I, J, mirror)
JOB_SLOTS = [(128, 128, True), (128, 128, True), (128, 128, True),
             (64, 64, True), (64, 128, False)]


BASE_DEG = 6  # exact V-recurrence up to here; higher rows are pure products


def _vchain(b):
    """Operands (m, n, k) with V_b = V_m*V_n - V_k, all indices < b."""
    if b % 2 == 0:
        return (b // 2 + 1, b // 2 - 1, 2)
    return ((b + 1) // 2, (b - 1) // 2, 1)


# ---------------------------------------------------------------------------
# numpy fallback (exact reference math) for input shapes/values outside the
# fast path.  kernel.py must be self-contained, so this re-implements the
# reference directly.


def _np_gelu(x):
    z = np.asarray(x, np.float64) / np.sqrt(2.0)
    try:
        from scipy.special import erf
        e = erf(z)
    except ImportError:
        import math
        e = np.vectorize(math.erf)(z)
    return np.asarray(x, np.float64) * (0.5 * (1.0 + e))


def _np_nonlinear(x, w1, b1, w2, b2):
    return (_np_gelu(np.asarray(x, F32) @ w1 + b1) @ w2 + b2).astype(F32)


def _np_gaussian(dist, etype, mul_w, bias_w, means, stds):
    mul = mul_w[etype]
    bias = bias_w[etype]
    x = mul * dist[..., None] + bias
    x = x - means
    std = np.abs(stds) + 1e-5
    return (np.exp(-0.5 * (x / std) ** 2) / (A_CONST * std)).astype(F32)


def _numpy_reference(pos, edge_types, protein_length, means, stds, mul_w, bias_w,
                     ow1, ob1, ow2, ob2, vw1, vb1, vw2, vb2):
    pos = np.asarray(pos, F32)
    Bv, Nv, _ = pos.shape
    P = int(protein_length)
    L = Nv - P
    Hv = ow2.shape[1]
    lig = pos[:, :L]
    prot = pos[:, L:]
    dlm_ll = lig[:, None, :, :] - lig[:, :, None, :]
    dlm_lp = lig[:, None, :, :] - prot[:, :, None, :]
    dist_ll = 1.0 / ((dlm_ll ** 2).sum(-1) + 1.0)
    dist_lp = 1.0 / ((dlm_lp ** 2).sum(-1) + 1.0)
    dlm_ll_h = _np_nonlinear(dlm_ll, vw1, vb1, vw2, vb2)
    dlm_pl_h = _np_nonlinear(-dlm_lp, vw1, vb1, vw2, vb2)
    dlm_lp_h = _np_nonlinear(dlm_lp, vw1, vb1, vw2, vb2)
    g_ll = _np_gaussian(dist_ll, edge_types[:, :L, :L], mul_w, bias_w, means, stds)
    ef_ll = _np_nonlinear(g_ll, ow1, ob1, ow2, ob2)
    g_lp = _np_gaussian(dist_lp, edge_types[:, L:, :L], mul_w, bias_w, means, stds)
    ef_lp = _np_nonlinear(g_lp, ow1, ob1, ow2, ob2)
    ef = np.zeros((Bv, Nv, Nv, Hv), F32)
    ef[:, :L, :L, :] = ef_ll + dlm_ll_h
    ef[:, L:, :L, :] = ef_lp + dlm_lp_h
    ef[:, :L, L:, :] = np.swapaxes(ef_lp + dlm_pl_h, 1, 2)
    return np.transpose(ef, (0, 3, 1, 2)).copy()


# ---------------------------------------------------------------------------
# host-side Chebyshev fit of the edge-feature path


def _vladder_fp16(d32):
    """Simulate the device fp16 basis ladder exactly: V-recurrence up to
    BASE_DEG, then products V_b = V_BASE * V_{b-BASE} for higher degrees."""
    D = DCHEB
    V = [None] * (D + 1)
    V[1] = (4.0 * d32.astype(F32) - 2.0).astype(F16)
    V[2] = ((V[1] * V[1]).astype(F16).astype(F32) - 2.0).astype(F16)
    for b in range(3, BASE_DEG + 1):
        m, n, k = _vchain(b)
        V[b] = ((V[m] * V[n]).astype(F16) - V[k]).astype(F16)
    for b in range(BASE_DEG + 1, D + 1):
        V[b] = (V[BASE_DEG] * V[b - BASE_DEG]).astype(F16)
    return np.stack(V[1:], 0)  # [D, M]


def _fit_ef_cheb(means, stds, ow1, ob1, ow2):
    """Least-squares fit of ef(d) (without ob2) on d in (0,1] against the
    exact fp16 device basis.  Returns co [D+1, 32] (row 0 = constant)."""
    M = 16384
    dgrid = (np.arange(M, dtype=np.float64) + 0.5) / M
    s = np.abs(stds) + 1e-5
    xg = (dgrid[:, None] - means) / s
    G = np.exp(-0.5 * xg * xg) / (A_CONST * s)
    f = _np_gelu(G @ ow1 + ob1) @ ow2       # [M, 32] float64
    Vd = _vladder_fp16(dgrid).astype(np.float64)          # [D, M]
    Bm = np.concatenate([np.ones((1, M)), Vd], axis=0).T  # [M, D+1]
    co, *_ = np.linalg.lstsq(Bm, f, rcond=None)           # [D+1, 32]
    return co, dgrid, f


def _fit_error(co, dgrid, f):
    cm = co[1:].astype(F16).astype(F32)     # [D, 32] as sent to device
    Vd = _vladder_fp16(dgrid)
    est = Vd.astype(F32).T @ cm + co[0][None, :].astype(F32)
    return float(np.abs(est - f).max())


# ---------------------------------------------------------------------------
# device program


_PROGRAM_CACHE = {}


def _build_program():
    """Build the SPMD Bass program (identical for all 8 cores)."""
    _ensure_concourse()
    import contextlib

    import concourse.bass as bass  # noqa: F401
    import concourse.tile as tile
    from concourse import bacc, mybir
    from concourse.tile import add_dep_helper

    dt = mybir.dt
    AF = mybir.ActivationFunctionType
    ALU = mybir.AluOpType

    D = DCHEB
    nc = bacc.Bacc("TRN2", target_bir_lowering=False, debug=False)

    def din(name, shape, dd=None):
        return nc.dram_tensor(name, list(shape), dd or dt.float32,
                              kind="ExternalInput").ap()

    CM = din("CM", (K, H), dt.float16)   # rows 0..D-1 = cheb c_b / 2
    V2 = din("V2", (K, H), dt.float16)   # vw2
    jin, jout, jscr = [], [], []
    for jidx, (I, J, mirror) in enumerate(JOB_SLOTS):
        NP = I * J
        jin.append({
            "fg": din(f"fg{jidx}", (5, I + J)),                    # gl | gr
            "tv": din(f"tv{jidx}", (K, 2 * I + J), dt.float16),    # tvI2 | tvJ
        })
        jout.append({"od": nc.dram_tensor(
            f"od{jidx}", [K, NP // 4], dt.float16, kind="ExternalOutput").ap()})
        jscr.append(nc.dram_tensor(
            f"bs{jidx}", [D, NP], dt.float16, kind="Internal").ap())

    def raw(inst):
        return inst.ins if hasattr(inst, "ins") else inst

    with tile.TileContext(nc) as tc:
        stack = contextlib.ExitStack()
        consts = stack.enter_context(tc.tile_pool(name="consts", bufs=1))
        vpool = stack.enter_context(tc.tile_pool(name="vpool", bufs=1))
        bpool = stack.enter_context(tc.tile_pool(name="bpool", bufs=2))
        bpoolS = stack.enter_context(tc.tile_pool(name="bpoolS", bufs=1))
        upool = stack.enter_context(tc.tile_pool(name="upool", bufs=7))
        hvpool = stack.enter_context(tc.tile_pool(name="hvpool", bufs=8))
        stpool = stack.enter_context(tc.tile_pool(name="stpool", bufs=4))
        psR = stack.enter_context(tc.tile_pool(name="psR", bufs=1, space="PSUM"))
        psO = stack.enter_context(tc.tile_pool(name="psO", bufs=6, space="PSUM"))

        # job0 inputs land first so its pipeline starts immediately
        sbj = [None] * len(JOB_SLOTS)
        for jidx in [0]:
            t = {}
            for kind in ("tv", "fg"):
                shp = list(jin[jidx][kind].shape)
                dd = dt.float16 if kind == "tv" else dt.float32
                t[kind] = consts.tile(shp, dd, name=f"jc_{kind}{jidx}")
                nc.sync.dma_start(out=t[kind][:, :], in_=jin[jidx][kind])
            sbj[jidx] = t
        CM_s = consts.tile([K, H], dt.float16, name="cm")
        nc.sync.dma_start(out=CM_s[:, :], in_=CM)
        V2_s = consts.tile([K, H], dt.float16, name="v2")
        nc.sync.dma_start(out=V2_s[:, :], in_=V2)
        for jidx in range(1, len(JOB_SLOTS)):
            t = {}
            for kind in ("fg", "tv"):
                shp = list(jin[jidx][kind].shape)
                dd = dt.float16 if kind == "tv" else dt.float32
                t[kind] = consts.tile(shp, dd, name=f"jc_{kind}{jidx}")
                nc.sync.dma_start(out=t[kind][:, :], in_=jin[jidx][kind])
            sbj[jidx] = t

        # ---- d = 1/(r^2+1) per job into one [128, 640] tile ---------------
        D_all = vpool.tile([K, 128 * len(JOB_SLOTS)], dt.float32, name="D_all")
        nc.gpsimd.memset(D_all[:, :], 0.0)
        # warmup activation: hoists the auto-inserted Gelu table load (1.3us)
        # into the head idle instead of delaying the first real gelu
        warm = vpool.tile([K, 1], dt.float16, name="warm")
        nc.scalar.activation(warm[:, :], D_all[:, 0:1], AF.Gelu, bias=0.0)

        def emit_recip(jidx):
            I, J, _ = JOB_SLOTS[jidx]
            joff = 128 * jidx
            fg = sbj[jidx]["fg"]
            pR = psR.tile([128, 128], dt.float32, tag="r", name="pR")
            nc.tensor.matmul(pR[:I, :J], fg[:, 0:I], fg[:, I:I + J],
                             start=True, stop=True)
            nc.vector.reciprocal(D_all[:I, joff:joff + J], pR[:I, :J])

        # ---- fp16 basis ladder --------------------------------------------
        # Exact V-recurrence (V_b = 2*T_b) up to BASE_DEG, then pure products
        # V_b = V_BASE * V_{b-BASE}: 29 DVE ops instead of 44, and O(log)
        # dependency depth.  Split in two column ranges: job0's 128 cols
        # first (unblocks the first basis DMA early), then the rest.
        NC = 128 * len(JOB_SLOTS)
        V_all = vpool.tile([K, D * NC], dt.float16, name="V_all")

        def ladder(c0, c1):
            w = c1 - c0

            def V(b):
                return V_all[:, (b - 1) * NC + c0:(b - 1) * NC + c1]

            nc.vector.tensor_scalar(V(1), D_all[:, c0:c1], 4.0, -2.0,
                                    ALU.mult, ALU.add)
            mt = vpool.tile([K, w], dt.float16, name=f"lm0_{c0}",
                            tag=f"lm0_{w}")
            nc.vector.tensor_mul(mt[:, :w], V(1), V(1))
            nc.vector.tensor_scalar(V(2), mt[:, :w], -2.0, None, ALU.add)
            for b in range(3, BASE_DEG + 1):
                m, n, k = _vchain(b)
                mm = vpool.tile([K, w], dt.float16, name=f"lm{b}_{c0}",
                                tag=f"lm{b % 2}_{w}")
                nc.vector.tensor_mul(mm[:, :w], V(m), V(n))
                nc.vector.tensor_sub(V(b), mm[:, :w], V(k))
            for b in range(BASE_DEG + 1, D + 1):
                nc.vector.tensor_mul(V(b), V(BASE_DEG), V(b - BASE_DEG))

        d1_insts = {}

        def emit_d1(jidx, i0=0, i1=None):
            I, J, _ = JOB_SLOTS[jidx]
            if i1 is None:
                i1 = I
            joff = 128 * jidx
            vsl = V_all[:, :].rearrange("p (b c) -> p b c", b=D,
                                        c=NC)[i0:i1, :, joff:joff + J]
            d1_insts[(jidx, i0)] = nc.sync.dma_start(
                out=jscr[jidx][:, i0 * J:i1 * J].rearrange(
                    "b (i j) -> i b j", i=i1 - i0, j=J),
                in_=vsl)

        Bts = {}

        def emit_d2(jidx, i0=0, i1=None):
            I, J, _ = JOB_SLOTS[jidx]
            NP = I * J
            if i1 is None:
                i1 = I
            if jidx in Bts:
                Bt = Bts[jidx]
            elif NP == 16384:
                # jobs 0..2 rotate two big buffers; jobs 3/4 get their own
                # smaller tiles so their loads never wait on buffer reuse
                Bt = bpool.tile([128, NP], dt.float16, tag="B",
                                name=f"Bt{jidx}")
            else:
                Bt = bpoolS.tile([128, NP], dt.float16, tag=f"Bs{jidx}",
                                 name=f"Bt{jidx}")
            d2 = nc.sync.dma_start(out=Bt[0:D, i0 * J:i1 * J],
                                   in_=jscr[jidx][:, i0 * J:i1 * J])
            add_dep_helper(raw(d2), raw(d1_insts[(jidx, i0)]), sync=True,
                           reason="bscratch RAW")
            Bts[jidx] = Bt

        def emit_ut(jidx, h, Ut, off, eng=None, sub=None):
            I, J, _ = JOB_SLOTS[jidx]
            iin = 2048 // J
            iw0 = h * iin
            w = 2048
            if sub is not None:
                iin //= 2
                iw0 += sub * iin
                off += sub * 1024
                w = 1024
            tv = sbj[jidx]["tv"]
            (eng or nc.vector).tensor_tensor(
                Ut[:, off:off + w].rearrange(
                    "p (ii jj j2) -> p ii jj j2", ii=iin, jj=J // 2, j2=2),
                tv[:, 2 * I:2 * I + J][:, None, :].broadcast_to(
                    [128, iin, J]).rearrange(
                    "p ii (jj j2) -> p ii jj j2", j2=2),
                tv[:, 2 * iw0:2 * (iw0 + iin)].rearrange(
                    "p (ii j2) -> p ii j2", j2=2)[:, :, None, :]
                .broadcast_to([128, iin, J // 2, 2]),
                ALU.subtract)

        # ---- schedule -----------------------------------------------------
        # 30 uniform 2048-pair half-group units.  Pool builds a unit's Ut in
        # 4.2us, DVE in 1.1us, ACT consumes one every 1.9us, so Pool covers
        # roughly every other slot while DVE runs the basis ladder pieces.
        # Drains lag their slot by 4 so the in-order DVE queue never parks
        # on an unfinished PSUM tile; output DMAs pair two consecutive
        # halves and follow the odd drain.
        SCHED = []
        for jidx in (0, 1, 2, 4, 3):
            I, J, _ = JOB_SLOTS[jidx]
            SCHED += [(jidx, h) for h in range(I * J // 2048)]
        POOL_SLOTS = {2, 4, 6, 9, 12, 14, 17, 19, 21, 23, 24, 26}

        pOs, sts = {}, {}

        def emit_proj(item, hvt, off):
            jidx, h = item
            Bt = Bts[jidx]
            pO = psO.tile([128, 512], dt.float32, tag="o", name="pO")
            for c in range(16):
                ch0 = h * 2048 + c * 128
                cs = 32 * c
                nc.tensor.matmul(pO[:, cs:cs + 32], Bt[0:D, ch0:ch0 + 128],
                                 CM_s[0:D, :], start=True, stop=False)
                nc.tensor.matmul(pO[:, cs:cs + 32],
                                 hvt[:, off + c * 128:off + (c + 1) * 128],
                                 V2_s[:, :], start=False, stop=True)
            pOs[item] = pO

        def emit_drain(item, on_act=False, split_out=False):
            jidx, h = item
            if h % 2 == 0:
                sts[jidx] = stpool.tile([128, 1024], dt.float16, tag="st",
                                        name="st")
            st = sts[jidx]
            sl = (h % 2) * 512
            if on_act:
                nc.scalar.activation(st[:, sl:sl + 512], pOs.pop(item)[:, :],
                                     AF.Copy)
            else:
                nc.vector.tensor_copy(st[:, sl:sl + 512], pOs.pop(item)[:, :])
            if split_out:
                # final pair: per-half outputs so the very last DMA is small
                nc.sync.dma_start(out=jout[jidx]["od"][:, h * 512:(h + 1) * 512],
                                  in_=st[:, sl:sl + 512])
            elif h % 2 == 1:
                nc.sync.dma_start(
                    out=jout[jidx]["od"][:, (h - 1) * 512:(h + 1) * 512],
                    in_=st[:, :])

        prehooks = {
            # between slot 0's gelu and its projection: job0 basis pipeline,
            # first half (pairs 0..8191) so the projections start early
            0: lambda: (emit_recip(0), ladder(0, 128),
                        emit_d1(0, 0, 64), emit_d2(0, 0, 64)),
        }
        hooks = {
            1: lambda: (emit_d1(0, 64, 128), emit_d2(0, 64, 128)),
            2: lambda: [emit_recip(j) for j in range(1, 5)],
            3: lambda: (ladder(128, 384), emit_d1(1), emit_d1(2), emit_d2(1)),
            8: lambda: (ladder(384, NC), emit_d1(4), emit_d1(3), emit_d2(2)),
            10: lambda: emit_d2(4),
            12: lambda: emit_d2(3),
        }
        for k, item in enumerate(SCHED):
            if k >= 3:
                emit_drain(SCHED[k - 3])
            eng = nc.gpsimd if k in POOL_SLOTS else nc.vector
            Ut = upool.tile([128, 2048], dt.float16, tag="u", name="Ut")
            hvt = hvpool.tile([128, 2048], dt.float16, tag="hv", name="hvt")
            emit_ut(*item, Ut=Ut, off=0, eng=eng)
            nc.scalar.activation(hvt[:, :], Ut[:, :], AF.Gelu, bias=0.0)
            if k in prehooks:
                prehooks[k]()
            emit_proj(item, hvt, 0)
            if k in hooks:
                hooks[k]()
        n = len(SCHED)
        for k in range(n, n + 3):
            # trailing drains stay off ACT: the gelu stream IS the makespan
            emit_drain(SCHED[k - 3], on_act=(k == n + 2))

        stack.close()

    nc.compile()
    return nc, {}


def _get_program():
    if "prog" not in _PROGRAM_CACHE:
        _PROGRAM_CACHE["prog"] = _build_program()
    return _PROGRAM_CACHE["prog"]


# ---------------------------------------------------------------------------
# host side


def _prep_core_inputs(core_jobs, pos, tvT_all, n2_all, consts):
    """Build the input map for one core."""
    m = dict(consts)
    for jidx, (b, i0, I, j0, J, mirror) in enumerate(core_jobs):
        p = pos[b]
        n2 = n2_all[b]
        tvT = tvT_all[b]
        fg = np.empty((5, I + J), F32)
        fg[0:3, :I] = -2.0 * p[i0:i0 + I].T
        fg[3, :I] = n2[i0:i0 + I]
        fg[4, :I] = 1.0
        fg[0:3, I:] = p[j0:j0 + J].T
        fg[3, I:] = 1.0
        fg[4, I:] = n2[j0:j0 + J] + 1.0
        tv = np.empty((K, 2 * I + J), F16)
        tv[:, 0:2 * I] = np.repeat(tvT[:, i0:i0 + I], 2, axis=1)
        tv[:, 2 * I:] = tvT[:, j0:j0 + J]
        m[f"fg{jidx}"] = np.ascontiguousarray(fg)
        m[f"tv{jidx}"] = np.ascontiguousarray(tv)
    return m


_RUNNER_CACHE = {}


def _get_runner(nc):
    """Compile (once) a jitted shard_map over the 8 cores with donated,
    device-side-created zero output buffers."""
    if "r" in _RUNNER_CACHE:
        return _RUNNER_CACHE["r"]
    _ensure_concourse()
    import jax
    import jax.numpy as jnp
    from jax.sharding import Mesh, NamedSharding, PartitionSpec
    from jax.experimental.shard_map import shard_map
    from concourse import mybir
    from concourse.bass2jax import (_bass_exec_p, install_neuronx_cc_hook,
                                    partition_id_tensor)

    install_neuronx_cc_hook()

    in_names, out_names, out_avals = [], [], []
    partition_name = (nc.partition_id_tensor.name
                      if nc.partition_id_tensor else None)
    for alloc in nc.m.functions[0].allocations:
        if not isinstance(alloc, mybir.MemoryLocationSet):
            continue
        name = alloc.memorylocations[0].name
        if alloc.kind == "ExternalInput":
            if name != partition_name:
                in_names.append(name)
        elif alloc.kind == "ExternalOutput":
            out_names.append(name)
            out_avals.append(jax.core.ShapedArray(
                tuple(alloc.tensor_shape), mybir.dt.np(alloc.dtype)))
    n_params = len(in_names)
    n_outs = len(out_avals)
    all_in_names = list(in_names) + list(out_names)
    if partition_name is not None:
        all_in_names.append(partition_name)

    def _body(*args):
        operands = list(args)
        if partition_name is not None:
            operands.append(partition_id_tensor())
        outs = _bass_exec_p.bind(
            *operands, out_avals=tuple(out_avals),
            in_names=tuple(all_in_names), out_names=tuple(out_names),
            lowering_input_output_aliases=(), sim_require_finite=True,
            sim_require_nnan=True, nc=nc)
        return tuple(outs)

    devices = jax.devices()[:8]
    mesh = Mesh(np.asarray(devices), ("core",))
    in_specs = (PartitionSpec("core"),) * (n_params + n_outs)
    out_specs = (PartitionSpec("core"),) * n_outs
    donate = tuple(range(n_params, n_params + n_outs))
    sharded = jax.jit(
        shard_map(_body, mesh=mesh, in_specs=in_specs, out_specs=out_specs,
                  check_rep=False),
        donate_argnums=donate, keep_unused=True)

    zshapes = [(8 * a.shape[0], *a.shape[1:]) for a in out_avals]
    zdtypes = [a.dtype for a in out_avals]
    mk = jax.jit(lambda: tuple(jnp.zeros(s, d)
                               for s, d in zip(zshapes, zdtypes)),
                 out_shardings=tuple(
                     NamedSharding(mesh, PartitionSpec("core"))
                     for _ in range(n_outs)))

    _RUNNER_CACHE["r"] = (sharded, mk, in_names, out_names, out_avals)
    return _RUNNER_CACHE["r"]


def _run_on_device(nc, in_maps):
    import jax

    sharded, mk, in_names, out_names, out_avals = _get_runner(nc)
    per_core = [[np.asarray(m[name]) for name in in_names] for m in in_maps]
    concat_in = [np.concatenate([per_core[c][i] for c in range(8)], axis=0)
                 for i in range(len(in_names))]
    out_arrs = jax.block_until_ready(sharded(*concat_in, *mk()))
    results = []
    for c in range(8):
        results.append({
            name: np.asarray(out_arrs[i]).reshape(8, *out_avals[i].shape)[c]
            for i, name in enumerate(out_names)})
    return results


def _decode_direct(arr, I, J):
    """[128, NP/4] fp16 pair-chunked slabs -> [H, I, J] fp32.

    arr[p, chunk*32 + h] holds pair n = chunk*128 + p, n = i*J + j.
    """
    NP = I * J
    nch = NP // 128
    a = arr.astype(F32).reshape(128, nch, 32)     # [p, chunk, h]
    a = a.transpose(1, 0, 2).reshape(NP, 32)      # pair-major [n, h]
    return a.reshape(I, J, 32).transpose(2, 0, 1)


def kernel(**inputs):
    pos = np.ascontiguousarray(np.asarray(inputs["pos"], F32))
    protein_length = int(np.asarray(inputs["protein_length"]))
    means = np.asarray(inputs["means"], np.float64)
    stds = np.asarray(inputs["stds"], np.float64)
    mul_w = np.asarray(inputs["mul_w"], F32)
    bias_w = np.asarray(inputs["bias_w"], F32)
    ow1 = np.asarray(inputs["ow1"], F32)
    ob1 = np.asarray(inputs["ob1"], F32)
    ow2 = np.asarray(inputs["ow2"], F32)
    ob2 = np.asarray(inputs["ob2"], F32)
    vw1 = np.asarray(inputs["vw1"], F32)
    vb1 = np.asarray(inputs["vb1"], F32)
    vw2 = np.asarray(inputs["vw2"], F32)
    vb2 = np.asarray(inputs["vb2"], F32)

    def _fallback():
        return _numpy_reference(pos, np.asarray(inputs["edge_types"]),
                                protein_length, means.astype(F32),
                                np.asarray(stds, F32), mul_w, bias_w, ow1, ob1,
                                ow2, ob2, vw1, vb1, vw2, vb2)

    fast_ok = (
        pos.shape == (B, N, 3)
        and protein_length == PLEN
        and means.shape == (K,)
        and ow1.shape == (K, K) and ow2.shape == (K, H)
        and vw1.shape == (3, K) and vw2.shape == (K, H)
        and np.all(mul_w == mul_w.reshape(-1)[0])
        and np.all(bias_w == bias_w.reshape(-1)[0])
        and np.all(vb1 == 0.0)
        and float(mul_w.reshape(-1)[0]) == 1.0
        and float(bias_w.reshape(-1)[0]) == 0.0
    )
    if not fast_ok:
        return _fallback()

    # host Chebyshev fit of the edge-feature path, with device-exact check
    co, dgrid, fref = _fit_ef_cheb(means, stds,
                                   ow1.astype(np.float64),
                                   ob1.astype(np.float64),
                                   ow2.astype(np.float64))
    if _fit_error(co, dgrid, fref) > 0.012:
        return _fallback()

    consts = {
        "CM": np.ascontiguousarray(co[1:].astype(F16)),             # [D, 32]
        "V2": np.ascontiguousarray(vw2.astype(F16)),
    }
    consts["CM"] = np.concatenate(
        [consts["CM"], np.zeros((K - DCHEB, H), F16)], axis=0)

    n2_all = (pos.astype(np.float64) ** 2).sum(-1).astype(F32)   # [B, N]
    tvT_all = np.stack([(pos[b] @ vw1).T for b in range(B)], 0).astype(F16)
    w3 = (vw1.astype(np.float64) @ vw2.astype(np.float64))       # [3, 32]
    sv_all = np.stack([(pos[b].astype(np.float64) @ w3).T.astype(F32)
                       for b in range(B)], 0)                    # [B, 32, N]
    outb = (ob2 + vb2 + co[0].astype(F32)).astype(F32)           # [32]

    cores = make_jobs()
    in_maps = [_prep_core_inputs(cores[c], pos, tvT_all, n2_all, consts)
               for c in range(8)]

    try:
        nc, meta = _get_program()
        try:
            results = _run_on_device(nc, in_maps)
        except Exception:
            _ensure_concourse()
            from concourse import bass_utils
            res = bass_utils.run_bass_kernel_spmd(nc, in_maps,
                                                  core_ids=list(range(8)))
            results = res.results
    except Exception:
        # No usable device path in this environment: fall back to the exact
        # host implementation so kernel() always returns a correct result.
        return _fallback()

    out = np.zeros((B, H, N, N), F32)
    bias3 = outb[:, None, None]
    for c in range(8):
        for jidx, (b, i0, I, j0, J, mirror) in enumerate(cores[c]):
            od = _decode_direct(results[c][f"od{jidx}"], I, J)
            out[b, :, i0:i0 + I, j0:j0 + J] = od + bias3
            if mirror:
                # mirror tile: gelu(-u) = gelu(u) - u gives
                # om[h,j,i] = od[h,i,j] - sv[h,j] + sv[h,i]
                sv = sv_all[b]
                out[b, :, j0:j0 + J, i0:i0 + I] = (
                    od.transpose(0, 2, 1) + bias3
                    - sv[:, j0:j0 + J, None] + sv[:, None, i0:i0 + I])
    return out


if __name__ == "__main__":
    nc, meta = _get_program()
    print("program built ok")


# revision 62
# speedup vs baseline: 1.0238x; 1.0079x over previous
"""Trainium2 Bass kernel for nn_DistanceBias (gnn_message_passing).

Math (derived from the reference):
  out[b,h,r,c] = ef(dist(r,c))[h] + vec(pos_c - pos_r)[h]   if r < L or c < L
               = 0                                           otherwise
with L = N - protein_length = 256 ligand nodes,
  dist(r,c) = 1/(|pos_r - pos_c|^2 + 1)  in (0, 1],
  ef(d)  = gelu(G(d) @ ow1 + ob1) @ ow2 + ob2,   G_k(d) = gaussian features
  vec(u) = gelu(u @ vw1 + vb1) @ vw2 + vb2.

Key structure exploited:
  * With constant mul_w/bias_w tables the whole edge-feature path ef(d) is a
    smooth scalar function of d in (0,1].  It is fit ONCE on the host as a
    degree-24 Chebyshev series; the device evaluates the basis per pair with a
    cheap fp16 product ladder (V_b = 2*T_b obeys V_{m+n} = V_m*V_n - V_|m-n|)
    and contracts basis x coefficients in the SAME PSUM matmul accumulation as
    the vector-path projection.  This removes the exp activation, the 128x128
    MLP matmuls and their gelu - the activation engine only runs the
    vector-path gelu (one column per pair).
  * dist is symmetric and (vb1 == 0) gelu(-u) = gelu(u) - u, so each unordered
    pair is computed ONCE; mirrors are reconstructed on the host from the
    rank-3 correction sv = pos @ (vw1 @ vw2).
  * The vector-path subtraction u = tvJ[:,j] - tvI[:,i] is built on DVE in the
    2x fp16 mode: the host sends tvI with every column DOUBLED so that all
    three operands end in a packed [1,2] access-pattern dim.
  * The Chebyshev basis is built in pair-tile layout [i, j] and transposed to
    matmul layout [basis, pair] with two large DMAs through a DRAM scratch
    (arbitrary DRAM access patterns make the reshape free).
  * The protein x protein quadrant (56% of output) is exactly zero and never
    touched on device.  Device outputs are fp16 blocked slabs; the host
    reshapes, adds the shared output bias (ob2 + vb2 + c0) and applies
    mirrors.

Work is split into an identical 5-job program per core (3 full 128x128 mirror
blocks + one 64x64 mirror quarter + one 64x128 ordered diag half = 61440
computed pairs per core, exactly 1/8 of the total).
"""

import os
import sys

import numpy as np

# ---------------------------------------------------------------------------
# problem constants (hardcoded per task instructions)
N = 1024
PLEN = 768
LIG = 256
K = 128
H = 32
B = 2
DCHEB = 18  # Chebyshev degree: rows V_1..V_D on device, c_0 folded into bias
A_CONST = (2.0 * 3.14159) ** 0.5  # matches reference PI

F32 = np.float32
F16 = np.float16


def _ensure_concourse():
    try:
        import concourse  # noqa: F401
        return
    except ImportError:
        pass
    for p in ("/opt/trn_rl_repo", "/root/.axon_site/_ro/trn_rl_repo"):
        if os.path.isdir(p) and p not in sys.path:
            sys.path.insert(0, p)
    import concourse  # noqa: F401


# ---------------------------------------------------------------------------
# job tables


def make_jobs():
    """Per-core job lists. Job = (batch, i0, I, j0, J, mirror).

    Fixed per-core structure (same shapes on every core so that a single
    SPMD program serves all 8 cores):
      jobs[0..2] : full 128x128 mirror blocks (LP region)
      jobs[3]    : 64x64 mirror quarter (LL off-diagonal block)
      jobs[4]    : 64x128 ordered diag half (LL diagonal blocks)
    """
    hd = [(0, 64, 0, 128), (64, 64, 0, 128), (128, 64, 128, 128), (192, 64, 128, 128)]
    qq = [(0, 64, 128, 64), (0, 64, 192, 64), (64, 64, 128, 64), (64, 64, 192, 64)]
    lp = []
    for b in range(B):
        for t in range(6):
            for jj in range(2):
                lp.append((b, 256 + 128 * t, 128, 128 * jj, 128, True))
    cores = []
    for c in range(8):
        b = c // 4
        jobs = list(lp[3 * c : 3 * c + 3])
        i0, I, j0, J = qq[c % 4]
        jobs.append((b, i0, I, j0, J, True))
        i0, I, j0, J = hd[c % 4]
        jobs.append((b, i0, I, j0, J, False))
        cores.append(jobs)
    return cores


# job slot shapes shared by the program on every core: (I, J, mirror)
JOB_SLOTS = [(128, 128, True), (128, 128, True), (128, 128, True),
             (64, 64, True), (64, 128, False)]


BASE_DEG = 6  # exact V-recurrence up to here; higher rows are pure products


def _vchain(b):
    """Operands (m, n, k) with V_b = V_m*V_n - V_k, all indices < b."""
    if b % 2 == 0:
        return (b // 2 + 1, b // 2 - 1, 2)
    return ((b + 1) // 2, (b - 1) // 2, 1)


# ---------------------------------------------------------------------------
# numpy fallback (exact reference math) for input shapes/values outside the
# fast path.  kernel.py must be self-contained, so this re-implements the
# reference directly.


def _np_gelu(x):
    z = np.asarray(x, np.float64) / np.sqrt(2.0)
    try:
        from scipy.special import erf
        e = erf(z)
    except ImportError:
        import math
        e = np.vectorize(math.erf)(z)
    return np.asarray(x, np.float64) * (0.5 * (1.0 + e))


def _np_nonlinear(x, w1, b1, w2, b2):
    return (_np_gelu(np.asarray(x, F32) @ w1 + b1) @ w2 + b2).astype(F32)


def _np_gaussian(dist, etype, mul_w, bias_w, means, stds):
    mul = mul_w[etype]
    bias = bias_w[etype]
    x = mul * dist[..., None] + bias
    x = x - means
    std = np.abs(stds) + 1e-5
    return (np.exp(-0.5 * (x / std) ** 2) / (A_CONST * std)).astype(F32)


def _numpy_reference(pos, edge_types, protein_length, means, stds, mul_w, bias_w,
                     ow1, ob1, ow2, ob2, vw1, vb1, vw2, vb2):
    pos = np.asarray(pos, F32)
    Bv, Nv, _ = pos.shape
    P = int(protein_length)
    L = Nv - P
    Hv = ow2.shape[1]
    lig = pos[:, :L]
    prot = pos[:, L:]
    dlm_ll = lig[:, None, :, :] - lig[:, :, None, :]
    dlm_lp = lig[:, None, :, :] - prot[:, :, None, :]
    dist_ll = 1.0 / ((dlm_ll ** 2).sum(-1) + 1.0)
    dist_lp = 1.0 / ((dlm_lp ** 2).sum(-1) + 1.0)
    dlm_ll_h = _np_nonlinear(dlm_ll, vw1, vb1, vw2, vb2)
    dlm_pl_h = _np_nonlinear(-dlm_lp, vw1, vb1, vw2, vb2)
    dlm_lp_h = _np_nonlinear(dlm_lp, vw1, vb1, vw2, vb2)
    g_ll = _np_gaussian(dist_ll, edge_types[:, :L, :L], mul_w, bias_w, means, stds)
    ef_ll = _np_nonlinear(g_ll, ow1, ob1, ow2, ob2)
    g_lp = _np_gaussian(dist_lp, edge_types[:, L:, :L], mul_w, bias_w, means, stds)
    ef_lp = _np_nonlinear(g_lp, ow1, ob1, ow2, ob2)
    ef = np.zeros((Bv, Nv, Nv, Hv), F32)
    ef[:, :L, :L, :] = ef_ll + dlm_ll_h
    ef[:, L:, :L, :] = ef_lp + dlm_lp_h
    ef[:, :L, L:, :] = np.swapaxes(ef_lp + dlm_pl_h, 1, 2)
    return np.transpose(ef, (0, 3, 1, 2)).copy()


# ---------------------------------------------------------------------------
# host-side Chebyshev fit of the edge-feature path


def _vladder_fp16(d32):
    """Simulate the device fp16 basis ladder exactly: V-recurrence up to
    BASE_DEG, then products V_b = V_BASE * V_{b-BASE} for higher degrees."""
    D = DCHEB
    V = [None] * (D + 1)
    V[1] = (4.0 * d32.astype(F32) - 2.0).astype(F16)
    V[2] = ((V[1] * V[1]).astype(F16).astype(F32) - 2.0).astype(F16)
    for b in range(3, BASE_DEG + 1):
        m, n, k = _vchain(b)
        V[b] = ((V[m] * V[n]).astype(F16) - V[k]).astype(F16)
    for b in range(BASE_DEG + 1, D + 1):
        V[b] = (V[BASE_DEG] * V[b - BASE_DEG]).astype(F16)
    return np.stack(V[1:], 0)  # [D, M]


def _fit_ef_cheb(means, stds, ow1, ob1, ow2):
    """Least-squares fit of ef(d) (without ob2) on d in (0,1] against the
    exact fp16 device basis.  Returns co [D+1, 32] (row 0 = constant)."""
    M = 16384
    dgrid = (np.arange(M, dtype=np.float64) + 0.5) / M
    s = np.abs(stds) + 1e-5
    xg = (dgrid[:, None] - means) / s
    G = np.exp(-0.5 * xg * xg) / (A_CONST * s)
    f = _np_gelu(G @ ow1 + ob1) @ ow2       # [M, 32] float64
    Vd = _vladder_fp16(dgrid).astype(np.float64)          # [D, M]
    Bm = np.concatenate([np.ones((1, M)), Vd], axis=0).T  # [M, D+1]
    co, *_ = np.linalg.lstsq(Bm, f, rcond=None)           # [D+1, 32]
    return co, dgrid, f


def _fit_error(co, dgrid, f):
    cm = co[1:].astype(F16).astype(F32)     # [D, 32] as sent to device
    Vd = _vladder_fp16(dgrid)
    est = Vd.astype(F32).T @ cm + co[0][None, :].astype(F32)
    return float(np.abs(est - f).max())


# ---------------------------------------------------------------------------
# device program


_PROGRAM_CACHE = {}


def _build_program():
    """Build the SPMD Bass program (identical for all 8 cores)."""
    _ensure_concourse()
    import contextlib

    import concourse.bass as bass  # noqa: F401
    import concourse.tile as tile
    from concourse import bacc, mybir
    from concourse.tile import add_dep_helper

    dt = mybir.dt
    AF = mybir.ActivationFunctionType
    ALU = mybir.AluOpType

    D = DCHEB
    nc = bacc.Bacc("TRN2", target_bir_lowering=False, debug=False)

    def din(name, shape, dd=None):
        return nc.dram_tensor(name, list(shape), dd or dt.float32,
                              kind="ExternalInput").ap()

    CM = din("CM", (K, H), dt.float16)   # rows 0..D-1 = cheb c_b / 2
    V2 = din("V2", (K, H), dt.float16)   # vw2
    jin, jout, jscr = [], [], []
    for jidx, (I, J, mirror) in enumerate(JOB_SLOTS):
        NP = I * J
        jin.append({
            "fg": din(f"fg{jidx}", (5, I + J)),                    # gl | gr
            "tv": din(f"tv{jidx}", (K, 2 * I + J), dt.float16),    # tvI2 | tvJ
        })
        jout.append({"od": nc.dram_tensor(
            f"od{jidx}", [K, NP // 4], dt.float16, kind="ExternalOutput").ap()})
        jscr.append(nc.dram_tensor(
            f"bs{jidx}", [D, NP], dt.float16, kind="Internal").ap())

    def raw(inst):
        return inst.ins if hasattr(inst, "ins") else inst

    with tile.TileContext(nc) as tc:
        stack = contextlib.ExitStack()
        consts = stack.enter_context(tc.tile_pool(name="consts", bufs=1))
        vpool = stack.enter_context(tc.tile_pool(name="vpool", bufs=1))
        bpool = stack.enter_context(tc.tile_pool(name="bpool", bufs=2))
        bpoolS = stack.enter_context(tc.tile_pool(name="bpoolS", bufs=1))
        upool = stack.enter_context(tc.tile_pool(name="upool", bufs=7))
        hvpool = stack.enter_context(tc.tile_pool(name="hvpool", bufs=8))
        stpool = stack.enter_context(tc.tile_pool(name="stpool", bufs=4))
        psR = stack.enter_context(tc.tile_pool(name="psR", bufs=1, space="PSUM"))
        psO = stack.enter_context(tc.tile_pool(name="psO", bufs=6, space="PSUM"))

        # job0 inputs land first so its pipeline starts immediately
        sbj = [None] * len(JOB_SLOTS)
        for jidx in [0]:
            t = {}
            for kind in ("tv", "fg"):
                shp = list(jin[jidx][kind].shape)
                dd = dt.float16 if kind == "tv" else dt.float32
                t[kind] = consts.tile(shp, dd, name=f"jc_{kind}{jidx}")
                nc.sync.dma_start(out=t[kind][:, :], in_=jin[jidx][kind])
            sbj[jidx] = t
        CM_s = consts.tile([K, H], dt.float16, name="cm")
        nc.sync.dma_start(out=CM_s[:, :], in_=CM)
        V2_s = consts.tile([K, H], dt.float16, name="v2")
        nc.sync.dma_start(out=V2_s[:, :], in_=V2)
        for jidx in range(1, len(JOB_SLOTS)):
            t = {}
            for kind in ("fg", "tv"):
                shp = list(jin[jidx][kind].shape)
                dd = dt.float16 if kind == "tv" else dt.float32
                t[kind] = consts.tile(shp, dd, name=f"jc_{kind}{jidx}")
                nc.sync.dma_start(out=t[kind][:, :], in_=jin[jidx][kind])
            sbj[jidx] = t

        # ---- d = 1/(r^2+1) per job into one [128, 640] tile ---------------
        D_all = vpool.tile([K, 128 * len(JOB_SLOTS)], dt.float32, name="D_all")
        nc.gpsimd.memset(D_all[:, :], 0.0)
        # warmup activation: hoists the auto-inserted Gelu table load (1.3us)
        # into the head idle instead of delaying the first real gelu
        warm = vpool.tile([K, 1], dt.float16, name="warm")
        nc.scalar.activation(warm[:, :], D_all[:, 0:1], AF.Gelu, bias=0.0)

        def emit_recip(jidx):
            I, J, _ = JOB_SLOTS[jidx]
            joff = 128 * jidx
            fg = sbj[jidx]["fg"]
            pR = psR.tile([128, 128], dt.float32, tag="r", name="pR")
            nc.tensor.matmul(pR[:I, :J], fg[:, 0:I], fg[:, I:I + J],
                             start=True, stop=True)
            nc.vector.reciprocal(D_all[:I, joff:joff + J], pR[:I, :J])

        # ---- fp16 basis ladder --------------------------------------------
        # Exact V-recurrence (V_b = 2*T_b) up to BASE_DEG, then pure products
        # V_b = V_BASE * V_{b-BASE}: 29 DVE ops instead of 44, and O(log)
        # dependency depth.  Split in two column ranges: job0's 128 cols
        # first (unblocks the first basis DMA early), then the rest.
        NC = 128 * len(JOB_SLOTS)
        V_all = vpool.tile([K, D * NC], dt.float16, name="V_all")

        def ladder(c0, c1):
            w = c1 - c0

            def V(b):
                return V_all[:, (b - 1) * NC + c0:(b - 1) * NC + c1]

            nc.vector.tensor_scalar(V(1), D_all[:, c0:c1], 4.0, -2.0,
                                    ALU.mult, ALU.add)
            mt = vpool.tile([K, w], dt.float16, name=f"lm0_{c0}",
                            tag=f"lm0_{w}")
            nc.vector.tensor_mul(mt[:, :w], V(1), V(1))
            nc.vector.tensor_scalar(V(2), mt[:, :w], -2.0, None, ALU.add)
            for b in range(3, BASE_DEG + 1):
                m, n, k = _vchain(b)
                mm = vpool.tile([K, w], dt.float16, name=f"lm{b}_{c0}",
                                tag=f"lm{b % 2}_{w}")
                nc.vector.tensor_mul(mm[:, :w], V(m), V(n))
                nc.vector.tensor_sub(V(b), mm[:, :w], V(k))
            for b in range(BASE_DEG + 1, D + 1):
                nc.vector.tensor_mul(V(b), V(BASE_DEG), V(b - BASE_DEG))

        d1_insts = {}

        def emit_d1(jidx, i0=0, i1=None):
            I, J, _ = JOB_SLOTS[jidx]
            if i1 is None:
                i1 = I
            joff = 128 * jidx
            vsl = V_all[:, :].rearrange("p (b c) -> p b c", b=D,
                                        c=NC)[i0:i1, :, joff:joff + J]
            d1_insts[(jidx, i0)] = nc.sync.dma_start(
                out=jscr[jidx][:, i0 * J:i1 * J].rearrange(
                    "b (i j) -> i b j", i=i1 - i0, j=J),
                in_=vsl)

        Bts = {}

        def emit_d2(jidx, i0=0, i1=None):
            I, J, _ = JOB_SLOTS[jidx]
            NP = I * J
            if i1 is None:
                i1 = I
            if jidx in Bts:
                Bt = Bts[jidx]
            elif NP == 16384:
                # jobs 0..2 rotate two big buffers; jobs 3/4 get their own
                # smaller tiles so their loads never wait on buffer reuse
                Bt = bpool.tile([128, NP], dt.float16, tag="B",
                                name=f"Bt{jidx}")
            else:
                Bt = bpoolS.tile([128, NP], dt.float16, tag=f"Bs{jidx}",
                                 name=f"Bt{jidx}")
            d2 = nc.sync.dma_start(out=Bt[0:D, i0 * J:i1 * J],
                                   in_=jscr[jidx][:, i0 * J:i1 * J])
            add_dep_helper(raw(d2), raw(d1_insts[(jidx, i0)]), sync=True,
                           reason="bscratch RAW")
            Bts[jidx] = Bt

        def emit_ut(jidx, h, Ut, off, eng=None, sub=None):
            I, J, _ = JOB_SLOTS[jidx]
            iin = 2048 // J
            iw0 = h * iin
            w = 2048
            if sub is not None:
                iin //= 2
                iw0 += sub * iin
                off += sub * 1024
                w = 1024
            tv = sbj[jidx]["tv"]
            (eng or nc.vector).tensor_tensor(
                Ut[:, off:off + w].rearrange(
                    "p (ii jj j2) -> p ii jj j2", ii=iin, jj=J // 2, j2=2),
                tv[:, 2 * I:2 * I + J][:, None, :].broadcast_to(
                    [128, iin, J]).rearrange(
                    "p ii (jj j2) -> p ii jj j2", j2=2),
                tv[:, 2 * iw0:2 * (iw0 + iin)].rearrange(
                    "p (ii j2) -> p ii j2", j2=2)[:, :, None, :]
                .broadcast_to([128, iin, J // 2, 2]),
                ALU.subtract)

        # ---- schedule -----------------------------------------------------
        # 30 uniform 2048-pair half-group units.  Pool builds a unit's Ut in
        # 4.2us, DVE in 1.1us, ACT consumes one every 1.9us, so Pool covers
        # roughly every other slot while DVE runs the basis ladder pieces.
        # Drains lag their slot by 4 so the in-order DVE queue never parks
        # on an unfinished PSUM tile; output DMAs pair two consecutive
        # halves and follow the odd drain.
        SCHED = []
        for jidx in (0, 1, 2, 4, 3):
            I, J, _ = JOB_SLOTS[jidx]
            SCHED += [(jidx, h) for h in range(I * J // 2048)]
        POOL_SLOTS = {2, 4, 6, 9, 12, 14, 17, 19, 21, 23, 24, 26}

        pOs, sts = {}, {}

        def emit_proj(item, hvt, off):
            jidx, h = item
            Bt = Bts[jidx]
            pO = psO.tile([128, 512], dt.float32, tag="o", name="pO")
            for c in range(16):
                ch0 = h * 2048 + c * 128
                cs = 32 * c
                nc.tensor.matmul(pO[:, cs:cs + 32], Bt[0:D, ch0:ch0 + 128],
                                 CM_s[0:D, :], start=True, stop=False)
                nc.tensor.matmul(pO[:, cs:cs + 32],
                                 hvt[:, off + c * 128:off + (c + 1) * 128],
                                 V2_s[:, :], start=False, stop=True)
            pOs[item] = pO

        def emit_drain(item, on_act=False, split_out=False):
            jidx, h = item
            if h % 2 == 0:
                sts[jidx] = stpool.tile([128, 1024], dt.float16, tag="st",
                                        name="st")
            st = sts[jidx]
            sl = (h % 2) * 512
            if on_act:
                nc.scalar.activation(st[:, sl:sl + 512], pOs.pop(item)[:, :],
                                     AF.Copy)
            else:
                nc.vector.tensor_copy(st[:, sl:sl + 512], pOs.pop(item)[:, :])
            if split_out:
                # final pair: per-half outputs so the very last DMA is small
                nc.sync.dma_start(out=jout[jidx]["od"][:, h * 512:(h + 1) * 512],
                                  in_=st[:, sl:sl + 512])
            elif h % 2 == 1:
                nc.sync.dma_start(
                    out=jout[jidx]["od"][:, (h - 1) * 512:(h + 1) * 512],
                    in_=st[:, :])

        prehooks = {
            # between slot 0's gelu and its projection: job0 basis pipeline,
            # first half (pairs 0..8191) so the projections start early
            0: lambda: (emit_recip(0), ladder(0, 128),
                        emit_d1(0, 0, 64), emit_d2(0, 0, 64)),
        }
        hooks = {
            1: lambda: (emit_d1(0, 64, 128), emit_d2(0, 64, 128)),
            2: lambda: [emit_recip(j) for j in range(1, 5)],
            3: lambda: (ladder(128, 384), emit_d1(1), emit_d1(2), emit_d2(1)),
            8: lambda: (ladder(384, NC), emit_d1(4), emit_d1(3), emit_d2(2)),
            10: lambda: emit_d2(4),
            12: lambda: emit_d2(3),
        }
        for k, item in enumerate(SCHED):
            if k >= 3:
                emit_drain(SCHED[k - 3])
            eng = nc.gpsimd if k in POOL_SLOTS else nc.vector
            Ut = upool.tile([128, 2048], dt.float16, tag="u", name="Ut")
            hvt = hvpool.tile([128, 2048], dt.float16, tag="hv", name="hvt")
            emit_ut(*item, Ut=Ut, off=0, eng=eng)
            nc.scalar.activation(hvt[:, :], Ut[:, :], AF.Gelu, bias=0.0)
            if k in prehooks:
                prehooks[k]()
            emit_proj(item, hvt, 0)
            if k in hooks:
                hooks[k]()
        n = len(SCHED)
        for k in range(n, n + 3):
            # trailing drains stay off ACT: the gelu stream IS the makespan
            emit_drain(SCHED[k - 3], on_act=(k == n + 2))

        stack.close()

    nc.compile()
    return nc, {}


def _get_program():
    if "prog" not in _PROGRAM_CACHE:
        _PROGRAM_CACHE["prog"] = _build_program()
    return _PROGRAM_CACHE["prog"]


# ---------------------------------------------------------------------------
# host side


def _prep_core_inputs(core_jobs, pos, tvT_all, n2_all, consts):
    """Build the input map for one core."""
    m = dict(consts)
    for jidx, (b, i0, I, j0, J, mirror) in enumerate(core_jobs):
        p = pos[b]
        n2 = n2_all[b]
        tvT = tvT_all[b]
        fg = np.empty((5, I + J), F32)
        fg[0:3, :I] = -2.0 * p[i0:i0 + I].T
        fg[3, :I] = n2[i0:i0 + I]
        fg[4, :I] = 1.0
        fg[0:3, I:] = p[j0:j0 + J].T
        fg[3, I:] = 1.0
        fg[4, I:] = n2[j0:j0 + J] + 1.0
        tv = np.empty((K, 2 * I + J), F16)
        tv[:, 0:2 * I] = np.repeat(tvT[:, i0:i0 + I], 2, axis=1)
        tv[:, 2 * I:] = tvT[:, j0:j0 + J]
        m[f"fg{jidx}"] = np.ascontiguousarray(fg)
        m[f"tv{jidx}"] = np.ascontiguousarray(tv)
    return m


_RUNNER_CACHE = {}


def _get_runner(nc):
    """Compile (once) a jitted shard_map over the 8 cores with donated,
    device-side-created zero output buffers."""
    if "r" in _RUNNER_CACHE:
        return _RUNNER_CACHE["r"]
    _ensure_concourse()
    import jax
    import jax.numpy as jnp
    from jax.sharding import Mesh, NamedSharding, PartitionSpec
    from jax.experimental.shard_map import shard_map
    from concourse import mybir
    from concourse.bass2jax import (_bass_exec_p, install_neuronx_cc_hook,
                                    partition_id_tensor)

    install_neuronx_cc_hook()

    in_names, out_names, out_avals = [], [], []
    partition_name = (nc.partition_id_tensor.name
                      if nc.partition_id_tensor else None)
    for alloc in nc.m.functions[0].allocations:
        if not isinstance(alloc, mybir.MemoryLocationSet):
            continue
        name = alloc.memorylocations[0].name
        if alloc.kind == "ExternalInput":
            if name != partition_name:
                in_names.append(name)
        elif alloc.kind == "ExternalOutput":
            out_names.append(name)
            out_avals.append(jax.core.ShapedArray(
                tuple(alloc.tensor_shape), mybir.dt.np(alloc.dtype)))
    n_params = len(in_names)
    n_outs = len(out_avals)
    all_in_names = list(in_names) + list(out_names)
    if partition_name is not None:
        all_in_names.append(partition_name)

    def _body(*args):
        operands = list(args)
        if partition_name is not None:
            operands.append(partition_id_tensor())
        outs = _bass_exec_p.bind(
            *operands, out_avals=tuple(out_avals),
            in_names=tuple(all_in_names), out_names=tuple(out_names),
            lowering_input_output_aliases=(), sim_require_finite=True,
            sim_require_nnan=True, nc=nc)
        return tuple(outs)

    devices = jax.devices()[:8]
    mesh = Mesh(np.asarray(devices), ("core",))
    in_specs = (PartitionSpec("core"),) * (n_params + n_outs)
    out_specs = (PartitionSpec("core"),) * n_outs
    donate = tuple(range(n_params, n_params + n_outs))
    sharded = jax.jit(
        shard_map(_body, mesh=mesh, in_specs=in_specs, out_specs=out_specs,
                  check_rep=False),
        donate_argnums=donate, keep_unused=True)

    zshapes = [(8 * a.shape[0], *a.shape[1:]) for a in out_avals]
    zdtypes = [a.dtype for a in out_avals]
    mk = jax.jit(lambda: tuple(jnp.zeros(s, d)
                               for s, d in zip(zshapes, zdtypes)),
                 out_shardings=tuple(
                     NamedSharding(mesh, PartitionSpec("core"))
                     for _ in range(n_outs)))

    _RUNNER_CACHE["r"] = (sharded, mk, in_names, out_names, out_avals)
    return _RUNNER_CACHE["r"]


def _run_on_device(nc, in_maps):
    import jax

    sharded, mk, in_names, out_names, out_avals = _get_runner(nc)
    per_core = [[np.asarray(m[name]) for name in in_names] for m in in_maps]
    concat_in = [np.concatenate([per_core[c][i] for c in range(8)], axis=0)
                 for i in range(len(in_names))]
    out_arrs = jax.block_until_ready(sharded(*concat_in, *mk()))
    results = []
    for c in range(8):
        results.append({
            name: np.asarray(out_arrs[i]).reshape(8, *out_avals[i].shape)[c]
            for i, name in enumerate(out_names)})
    return results


def _decode_direct(arr, I, J):
    """[128, NP/4] fp16 pair-chunked slabs -> [H, I, J] fp32.

    arr[p, chunk*32 + h] holds pair n = chunk*128 + p, n = i*J + j.
    """
    NP = I * J
    nch = NP // 128
    a = arr.astype(F32).reshape(128, nch, 32)     # [p, chunk, h]
    a = a.transpose(1, 0, 2).reshape(NP, 32)      # pair-major [n, h]
    return a.reshape(I, J, 32).transpose(2, 0, 1)


def kernel(**inputs):
    pos = np.ascontiguousarray(np.asarray(inputs["pos"], F32))
    protein_length = int(np.asarray(inputs["protein_length"]))
    means = np.asarray(inputs["means"], np.float64)
    stds = np.asarray(inputs["stds"], np.float64)
    mul_w = np.asarray(inputs["mul_w"], F32)
    bias_w = np.asarray(inputs["bias_w"], F32)
    ow1 = np.asarray(inputs["ow1"], F32)
    ob1 = np.asarray(inputs["ob1"], F32)
    ow2 = np.asarray(inputs["ow2"], F32)
    ob2 = np.asarray(inputs["ob2"], F32)
    vw1 = np.asarray(inputs["vw1"], F32)
    vb1 = np.asarray(inputs["vb1"], F32)
    vw2 = np.asarray(inputs["vw2"], F32)
    vb2 = np.asarray(inputs["vb2"], F32)

    def _fallback():
        return _numpy_reference(pos, np.asarray(inputs["edge_types"]),
                                protein_length, means.astype(F32),
                                np.asarray(stds, F32), mul_w, bias_w, ow1, ob1,
                                ow2, ob2, vw1, vb1, vw2, vb2)

    fast_ok = (
        pos.shape == (B, N, 3)
        and protein_length == PLEN
        and means.shape == (K,)
        and ow1.shape == (K, K) and ow2.shape == (K, H)
        and vw1.shape == (3, K) and vw2.shape == (K, H)
        and np.all(mul_w == mul_w.reshape(-1)[0])
        and np.all(bias_w == bias_w.reshape(-1)[0])
        and np.all(vb1 == 0.0)
        and float(mul_w.reshape(-1)[0]) == 1.0
        and float(bias_w.reshape(-1)[0]) == 0.0
    )
    if not fast_ok:
        return _fallback()

    # host Chebyshev fit of the edge-feature path, with device-exact check
    co, dgrid, fref = _fit_ef_cheb(means, stds,
                                   ow1.astype(np.float64),
                                   ob1.astype(np.float64),
                                   ow2.astype(np.float64))
    if _fit_error(co, dgrid, fref) > 0.012:
        return _fallback()

    consts = {
        "CM": np.ascontiguousarray(co[1:].astype(F16)),             # [D, 32]
        "V2": np.ascontiguousarray(vw2.astype(F16)),
    }
    consts["CM"] = np.concatenate(
        [consts["CM"], np.zeros((K - DCHEB, H), F16)], axis=0)

    n2_all = (pos.astype(np.float64) ** 2).sum(-1).astype(F32)   # [B, N]
    tvT_all = np.stack([(pos[b] @ vw1).T for b in range(B)], 0).astype(F16)
    w3 = (vw1.astype(np.float64) @ vw2.astype(np.float64))       # [3, 32]
    sv_all = np.stack([(pos[b].astype(np.float64) @ w3).T.astype(F32)
                       for b in range(B)], 0)                    # [B, 32, N]
    outb = (ob2 + vb2 + co[0].astype(F32)).astype(F32)           # [32]

    cores = make_jobs()
    in_maps = [_prep_core_inputs(cores[c], pos, tvT_all, n2_all, consts)
               for c in range(8)]

    try:
        nc, meta = _get_program()
        try:
            results = _run_on_device(nc, in_maps)
        except Exception:
            _ensure_concourse()
            from concourse import bass_utils
            res = bass_utils.run_bass_kernel_spmd(nc, in_maps,
                                                  core_ids=list(range(8)))
            results = res.results
    except Exception:
        # No usable device path in this environment: fall back to the exact
        # host implementation so kernel() always returns a correct result.
        return _fallback()

    out = np.zeros((B, H, N, N), F32)
    bias3 = outb[:, None, None]
    for c in range(8):
        for jidx, (b, i0, I, j0, J, mirror) in enumerate(cores[c]):
            od = _decode_direct(results[c][f"od{jidx}"], I, J)
            out[b, :, i0:i0 + I, j0:j0 + J] = od + bias3
            if mirror:
                # mirror tile: gelu(-u) = gelu(u) - u gives
                # om[h,j,i] = od[h,i,j] - sv[h,j] + sv[h,i]
                sv = sv_all[b]
                out[b, :, j0:j0 + J, i0:i0 + I] = (
                    od.transpose(0, 2, 1) + bias3
                    - sv[:, j0:j0 + J, None] + sv[:, None, i0:i0 + I])
    return out


if __name__ == "__main__":
    nc, meta = _get_program()
    print("program built ok")


# revision 63
# speedup vs baseline: 1.0278x; 1.0039x over previous
"""Trainium2 Bass kernel for nn_DistanceBias (gnn_message_passing).

Math (derived from the reference):
  out[b,h,r,c] = ef(dist(r,c))[h] + vec(pos_c - pos_r)[h]   if r < L or c < L
               = 0                                           otherwise
with L = N - protein_length = 256 ligand nodes,
  dist(r,c) = 1/(|pos_r - pos_c|^2 + 1)  in (0, 1],
  ef(d)  = gelu(G(d) @ ow1 + ob1) @ ow2 + ob2,   G_k(d) = gaussian features
  vec(u) = gelu(u @ vw1 + vb1) @ vw2 + vb2.

Key structure exploited:
  * With constant mul_w/bias_w tables the whole edge-feature path ef(d) is a
    smooth scalar function of d in (0,1].  It is fit ONCE on the host as a
    degree-24 Chebyshev series; the device evaluates the basis per pair with a
    cheap fp16 product ladder (V_b = 2*T_b obeys V_{m+n} = V_m*V_n - V_|m-n|)
    and contracts basis x coefficients in the SAME PSUM matmul accumulation as
    the vector-path projection.  This removes the exp activation, the 128x128
    MLP matmuls and their gelu - the activation engine only runs the
    vector-path gelu (one column per pair).
  * dist is symmetric and (vb1 == 0) gelu(-u) = gelu(u) - u, so each unordered
    pair is computed ONCE; mirrors are reconstructed on the host from the
    rank-3 correction sv = pos @ (vw1 @ vw2).
  * The vector-path subtraction u = tvJ[:,j] - tvI[:,i] is built on DVE in the
    2x fp16 mode: the host sends tvI with every column DOUBLED so that all
    three operands end in a packed [1,2] access-pattern dim.
  * The Chebyshev basis is built in pair-tile layout [i, j] and transposed to
    matmul layout [basis, pair] with two large DMAs through a DRAM scratch
    (arbitrary DRAM access patterns make the reshape free).
  * The protein x protein quadrant (56% of output) is exactly zero and never
    touched on device.  Device outputs are fp16 blocked slabs; the host
    reshapes, adds the shared output bias (ob2 + vb2 + c0) and applies
    mirrors.

Work is split into an identical 5-job program per core (3 full 128x128 mirror
blocks + one 64x64 mirror quarter + one 64x128 ordered diag half = 61440
computed pairs per core, exactly 1/8 of the total).
"""

import os
import sys

import numpy as np

# ---------------------------------------------------------------------------
# problem constants (hardcoded per task instructions)
N = 1024
PLEN = 768
LIG = 256
K = 128
H = 32
B = 2
DCHEB = 17  # Chebyshev degree: rows V_1..V_D on device, c_0 folded into bias
A_CONST = (2.0 * 3.14159) ** 0.5  # matches reference PI

F32 = np.float32
F16 = np.float16


def _ensure_concourse():
    try:
        import concourse  # noqa: F401
        return
    except ImportError:
        pass
    for p in ("/opt/trn_rl_repo", "/root/.axon_site/_ro/trn_rl_repo"):
        if os.path.isdir(p) and p not in sys.path:
            sys.path.insert(0, p)
    import concourse  # noqa: F401


# ---------------------------------------------------------------------------
# job tables


def make_jobs():
    """Per-core job lists. Job = (batch, i0, I, j0, J, mirror).

    Fixed per-core structure (same shapes on every core so that a single
    SPMD program serves all 8 cores):
      jobs[0..2] : full 128x128 mirror blocks (LP region)
      jobs[3]    : 64x64 mirror quarter (LL off-diagonal block)
      jobs[4]    : 64x128 ordered diag half (LL diagonal blocks)
    """
    hd = [(0, 64, 0, 128), (64, 64, 0, 128), (128, 64, 128, 128), (192, 64, 128, 128)]
    qq = [(0, 64, 128, 64), (0, 64, 192, 64), (64, 64, 128, 64), (64, 64, 192, 64)]
    lp = []
    for b in range(B):
        for t in range(6):
            for jj in range(2):
                lp.append((b, 256 + 128 * t, 128, 128 * jj, 128, True))
    cores = []
    for c in range(8):
        b = c // 4
        jobs = list(lp[3 * c : 3 * c + 3])
        i0, I, j0, J = qq[c % 4]
        jobs.append((b, i0, I, j0, J, True))
        i0, I, j0, J = hd[c % 4]
        jobs.append((b, i0, I, j0, J, False))
        cores.append(jobs)
    return cores


# job slot shapes shared by the program on every core: (I, J, mirror)
JOB_SLOTS = [(128, 128, True), (128, 128, True), (128, 128, True),
             (64, 64, True), (64, 128, False)]


BASE_DEG = 6  # exact V-recurrence up to here; higher rows are pure products


def _vchain(b):
    """Operands (m, n, k) with V_b = V_m*V_n - V_k, all indices < b."""
    if b % 2 == 0:
        return (b // 2 + 1, b // 2 - 1, 2)
    return ((b + 1) // 2, (b - 1) // 2, 1)


# ---------------------------------------------------------------------------
# numpy fallback (exact reference math) for input shapes/values outside the
# fast path.  kernel.py must be self-contained, so this re-implements the
# reference directly.


def _np_gelu(x):
    z = np.asarray(x, np.float64) / np.sqrt(2.0)
    try:
        from scipy.special import erf
        e = erf(z)
    except ImportError:
        import math
        e = np.vectorize(math.erf)(z)
    return np.asarray(x, np.float64) * (0.5 * (1.0 + e))


def _np_nonlinear(x, w1, b1, w2, b2):
    return (_np_gelu(np.asarray(x, F32) @ w1 + b1) @ w2 + b2).astype(F32)


def _np_gaussian(dist, etype, mul_w, bias_w, means, stds):
    mul = mul_w[etype]
    bias = bias_w[etype]
    x = mul * dist[..., None] + bias
    x = x - means
    std = np.abs(stds) + 1e-5
    return (np.exp(-0.5 * (x / std) ** 2) / (A_CONST * std)).astype(F32)


def _numpy_reference(pos, edge_types, protein_length, means, stds, mul_w, bias_w,
                     ow1, ob1, ow2, ob2, vw1, vb1, vw2, vb2):
    pos = np.asarray(pos, F32)
    Bv, Nv, _ = pos.shape
    P = int(protein_length)
    L = Nv - P
    Hv = ow2.shape[1]
    lig = pos[:, :L]
    prot = pos[:, L:]
    dlm_ll = lig[:, None, :, :] - lig[:, :, None, :]
    dlm_lp = lig[:, None, :, :] - prot[:, :, None, :]
    dist_ll = 1.0 / ((dlm_ll ** 2).sum(-1) + 1.0)
    dist_lp = 1.0 / ((dlm_lp ** 2).sum(-1) + 1.0)
    dlm_ll_h = _np_nonlinear(dlm_ll, vw1, vb1, vw2, vb2)
    dlm_pl_h = _np_nonlinear(-dlm_lp, vw1, vb1, vw2, vb2)
    dlm_lp_h = _np_nonlinear(dlm_lp, vw1, vb1, vw2, vb2)
    g_ll = _np_gaussian(dist_ll, edge_types[:, :L, :L], mul_w, bias_w, means, stds)
    ef_ll = _np_nonlinear(g_ll, ow1, ob1, ow2, ob2)
    g_lp = _np_gaussian(dist_lp, edge_types[:, L:, :L], mul_w, bias_w, means, stds)
    ef_lp = _np_nonlinear(g_lp, ow1, ob1, ow2, ob2)
    ef = np.zeros((Bv, Nv, Nv, Hv), F32)
    ef[:, :L, :L, :] = ef_ll + dlm_ll_h
    ef[:, L:, :L, :] = ef_lp + dlm_lp_h
    ef[:, :L, L:, :] = np.swapaxes(ef_lp + dlm_pl_h, 1, 2)
    return np.transpose(ef, (0, 3, 1, 2)).copy()


# ---------------------------------------------------------------------------
# host-side Chebyshev fit of the edge-feature path


def _vladder_fp16(d32):
    """Simulate the device fp16 basis ladder exactly: V-recurrence up to
    BASE_DEG, then products V_b = V_BASE * V_{b-BASE} for higher degrees."""
    D = DCHEB
    V = [None] * (D + 1)
    V[1] = (4.0 * d32.astype(F32) - 2.0).astype(F16)
    V[2] = ((V[1] * V[1]).astype(F16).astype(F32) - 2.0).astype(F16)
    for b in range(3, BASE_DEG + 1):
        m, n, k = _vchain(b)
        V[b] = ((V[m] * V[n]).astype(F16) - V[k]).astype(F16)
    for b in range(BASE_DEG + 1, D + 1):
        V[b] = (V[BASE_DEG] * V[b - BASE_DEG]).astype(F16)
    return np.stack(V[1:], 0)  # [D, M]


def _fit_ef_cheb(means, stds, ow1, ob1, ow2):
    """Least-squares fit of ef(d) (without ob2) on d in (0,1] against the
    exact fp16 device basis.  Returns co [D+1, 32] (row 0 = constant)."""
    M = 16384
    dgrid = (np.arange(M, dtype=np.float64) + 0.5) / M
    s = np.abs(stds) + 1e-5
    xg = (dgrid[:, None] - means) / s
    G = np.exp(-0.5 * xg * xg) / (A_CONST * s)
    f = _np_gelu(G @ ow1 + ob1) @ ow2       # [M, 32] float64
    Vd = _vladder_fp16(dgrid).astype(np.float64)          # [D, M]
    Bm = np.concatenate([np.ones((1, M)), Vd], axis=0).T  # [M, D+1]
    co, *_ = np.linalg.lstsq(Bm, f, rcond=None)           # [D+1, 32]
    return co, dgrid, f


def _fit_error(co, dgrid, f):
    cm = co[1:].astype(F16).astype(F32)     # [D, 32] as sent to device
    Vd = _vladder_fp16(dgrid)
    est = Vd.astype(F32).T @ cm + co[0][None, :].astype(F32)
    return float(np.abs(est - f).max())


# ---------------------------------------------------------------------------
# device program


_PROGRAM_CACHE = {}


def _build_program():
    """Build the SPMD Bass program (identical for all 8 cores)."""
    _ensure_concourse()
    import contextlib

    import concourse.bass as bass  # noqa: F401
    import concourse.tile as tile
    from concourse import bacc, mybir
    from concourse.tile import add_dep_helper

    dt = mybir.dt
    AF = mybir.ActivationFunctionType
    ALU = mybir.AluOpType

    D = DCHEB
    nc = bacc.Bacc("TRN2", target_bir_lowering=False, debug=False)

    def din(name, shape, dd=None):
        return nc.dram_tensor(name, list(shape), dd or dt.float32,
                              kind="ExternalInput").ap()

    CM = din("CM", (K, H), dt.float16)   # rows 0..D-1 = cheb c_b / 2
    V2 = din("V2", (K, H), dt.float16)   # vw2
    jin, jout, jscr = [], [], []
    for jidx, (I, J, mirror) in enumerate(JOB_SLOTS):
        NP = I * J
        jin.append({
            "fg": din(f"fg{jidx}", (5, I + J)),                    # gl | gr
            "tv": din(f"tv{jidx}", (K, 2 * I + J), dt.float16),    # tvI2 | tvJ
        })
        jout.append({"od": nc.dram_tensor(
            f"od{jidx}", [K, NP // 4], dt.float16, kind="ExternalOutput").ap()})
        jscr.append(nc.dram_tensor(
            f"bs{jidx}", [D, NP], dt.float16, kind="Internal").ap())

    def raw(inst):
        return inst.ins if hasattr(inst, "ins") else inst

    with tile.TileContext(nc) as tc:
        stack = contextlib.ExitStack()
        consts = stack.enter_context(tc.tile_pool(name="consts", bufs=1))
        vpool = stack.enter_context(tc.tile_pool(name="vpool", bufs=1))
        bpool = stack.enter_context(tc.tile_pool(name="bpool", bufs=2))
        bpoolS = stack.enter_context(tc.tile_pool(name="bpoolS", bufs=1))
        upool = stack.enter_context(tc.tile_pool(name="upool", bufs=7))
        hvpool = stack.enter_context(tc.tile_pool(name="hvpool", bufs=8))
        stpool = stack.enter_context(tc.tile_pool(name="stpool", bufs=4))
        psR = stack.enter_context(tc.tile_pool(name="psR", bufs=1, space="PSUM"))
        psO = stack.enter_context(tc.tile_pool(name="psO", bufs=6, space="PSUM"))

        # job0 inputs land first so its pipeline starts immediately
        sbj = [None] * len(JOB_SLOTS)
        for jidx in [0]:
            t = {}
            for kind in ("tv", "fg"):
                shp = list(jin[jidx][kind].shape)
                dd = dt.float16 if kind == "tv" else dt.float32
                t[kind] = consts.tile(shp, dd, name=f"jc_{kind}{jidx}")
                nc.sync.dma_start(out=t[kind][:, :], in_=jin[jidx][kind])
            sbj[jidx] = t
        CM_s = consts.tile([K, H], dt.float16, name="cm")
        nc.sync.dma_start(out=CM_s[:, :], in_=CM)
        V2_s = consts.tile([K, H], dt.float16, name="v2")
        nc.sync.dma_start(out=V2_s[:, :], in_=V2)
        for jidx in range(1, len(JOB_SLOTS)):
            t = {}
            for kind in ("fg", "tv"):
                shp = list(jin[jidx][kind].shape)
                dd = dt.float16 if kind == "tv" else dt.float32
                t[kind] = consts.tile(shp, dd, name=f"jc_{kind}{jidx}")
                nc.sync.dma_start(out=t[kind][:, :], in_=jin[jidx][kind])
            sbj[jidx] = t

        # ---- d = 1/(r^2+1) per job into one [128, 640] tile ---------------
        D_all = vpool.tile([K, 128 * len(JOB_SLOTS)], dt.float32, name="D_all")
        nc.gpsimd.memset(D_all[:, :], 0.0)
        # warmup activation: hoists the auto-inserted Gelu table load (1.3us)
        # into the head idle instead of delaying the first real gelu
        warm = vpool.tile([K, 1], dt.float16, name="warm")
        nc.scalar.activation(warm[:, :], D_all[:, 0:1], AF.Gelu, bias=0.0)

        def emit_recip(jidx):
            I, J, _ = JOB_SLOTS[jidx]
            joff = 128 * jidx
            fg = sbj[jidx]["fg"]
            pR = psR.tile([128, 128], dt.float32, tag="r", name="pR")
            nc.tensor.matmul(pR[:I, :J], fg[:, 0:I], fg[:, I:I + J],
                             start=True, stop=True)
            nc.vector.reciprocal(D_all[:I, joff:joff + J], pR[:I, :J])

        # ---- fp16 basis ladder --------------------------------------------
        # Exact V-recurrence (V_b = 2*T_b) up to BASE_DEG, then pure products
        # V_b = V_BASE * V_{b-BASE}: 29 DVE ops instead of 44, and O(log)
        # dependency depth.  Split in two column ranges: job0's 128 cols
        # first (unblocks the first basis DMA early), then the rest.
        NC = 128 * len(JOB_SLOTS)
        V_all = vpool.tile([K, D * NC], dt.float16, name="V_all")

        def ladder(c0, c1):
            w = c1 - c0

            def V(b):
                return V_all[:, (b - 1) * NC + c0:(b - 1) * NC + c1]

            nc.vector.tensor_scalar(V(1), D_all[:, c0:c1], 4.0, -2.0,
                                    ALU.mult, ALU.add)
            mt = vpool.tile([K, w], dt.float16, name=f"lm0_{c0}",
                            tag=f"lm0_{w}")
            nc.vector.tensor_mul(mt[:, :w], V(1), V(1))
            nc.vector.tensor_scalar(V(2), mt[:, :w], -2.0, None, ALU.add)
            for b in range(3, BASE_DEG + 1):
                m, n, k = _vchain(b)
                mm = vpool.tile([K, w], dt.float16, name=f"lm{b}_{c0}",
                                tag=f"lm{b % 2}_{w}")
                nc.vector.tensor_mul(mm[:, :w], V(m), V(n))
                nc.vector.tensor_sub(V(b), mm[:, :w], V(k))
            for b in range(BASE_DEG + 1, D + 1):
                nc.vector.tensor_mul(V(b), V(BASE_DEG), V(b - BASE_DEG))

        d1_insts = {}

        def emit_d1(jidx, i0=0, i1=None):
            I, J, _ = JOB_SLOTS[jidx]
            if i1 is None:
                i1 = I
            joff = 128 * jidx
            vsl = V_all[:, :].rearrange("p (b c) -> p b c", b=D,
                                        c=NC)[i0:i1, :, joff:joff + J]
            d1_insts[(jidx, i0)] = nc.sync.dma_start(
                out=jscr[jidx][:, i0 * J:i1 * J].rearrange(
                    "b (i j) -> i b j", i=i1 - i0, j=J),
                in_=vsl)

        Bts = {}

        def emit_d2(jidx, i0=0, i1=None):
            I, J, _ = JOB_SLOTS[jidx]
            NP = I * J
            if i1 is None:
                i1 = I
            if jidx in Bts:
                Bt = Bts[jidx]
            elif NP == 16384:
                # jobs 0..2 rotate two big buffers; jobs 3/4 get their own
                # smaller tiles so their loads never wait on buffer reuse
                Bt = bpool.tile([128, NP], dt.float16, tag="B",
                                name=f"Bt{jidx}")
            else:
                Bt = bpoolS.tile([128, NP], dt.float16, tag=f"Bs{jidx}",
                                 name=f"Bt{jidx}")
            d2 = nc.sync.dma_start(out=Bt[0:D, i0 * J:i1 * J],
                                   in_=jscr[jidx][:, i0 * J:i1 * J])
            add_dep_helper(raw(d2), raw(d1_insts[(jidx, i0)]), sync=True,
                           reason="bscratch RAW")
            Bts[jidx] = Bt

        def emit_ut(jidx, h, Ut, off, eng=None, sub=None):
            I, J, _ = JOB_SLOTS[jidx]
            iin = 2048 // J
            iw0 = h * iin
            w = 2048
            if sub is not None:
                iin //= 2
                iw0 += sub * iin
                off += sub * 1024
                w = 1024
            tv = sbj[jidx]["tv"]
            (eng or nc.vector).tensor_tensor(
                Ut[:, off:off + w].rearrange(
                    "p (ii jj j2) -> p ii jj j2", ii=iin, jj=J // 2, j2=2),
                tv[:, 2 * I:2 * I + J][:, None, :].broadcast_to(
                    [128, iin, J]).rearrange(
                    "p ii (jj j2) -> p ii jj j2", j2=2),
                tv[:, 2 * iw0:2 * (iw0 + iin)].rearrange(
                    "p (ii j2) -> p ii j2", j2=2)[:, :, None, :]
                .broadcast_to([128, iin, J // 2, 2]),
                ALU.subtract)

        # ---- schedule -----------------------------------------------------
        # 30 uniform 2048-pair half-group units.  Pool builds a unit's Ut in
        # 4.2us, DVE in 1.1us, ACT consumes one every 1.9us, so Pool covers
        # roughly every other slot while DVE runs the basis ladder pieces.
        # Drains lag their slot by 4 so the in-order DVE queue never parks
        # on an unfinished PSUM tile; output DMAs pair two consecutive
        # halves and follow the odd drain.
        SCHED = []
        for jidx in (0, 1, 2, 4, 3):
            I, J, _ = JOB_SLOTS[jidx]
            SCHED += [(jidx, h) for h in range(I * J // 2048)]
        POOL_SLOTS = {2, 4, 6, 9, 12, 14, 17, 19, 21, 23, 24, 26}

        pOs, sts = {}, {}

        def emit_proj(item, hvt, off):
            jidx, h = item
            Bt = Bts[jidx]
            pO = psO.tile([128, 512], dt.float32, tag="o", name="pO")
            for c in range(16):
                ch0 = h * 2048 + c * 128
                cs = 32 * c
                nc.tensor.matmul(pO[:, cs:cs + 32], Bt[0:D, ch0:ch0 + 128],
                                 CM_s[0:D, :], start=True, stop=False)
                nc.tensor.matmul(pO[:, cs:cs + 32],
                                 hvt[:, off + c * 128:off + (c + 1) * 128],
                                 V2_s[:, :], start=False, stop=True)
            pOs[item] = pO

        def emit_drain(item, on_act=False, split_out=False):
            jidx, h = item
            if h % 2 == 0:
                sts[jidx] = stpool.tile([128, 1024], dt.float16, tag="st",
                                        name="st")
            st = sts[jidx]
            sl = (h % 2) * 512
            if on_act:
                nc.scalar.activation(st[:, sl:sl + 512], pOs.pop(item)[:, :],
                                     AF.Copy)
            else:
                nc.vector.tensor_copy(st[:, sl:sl + 512], pOs.pop(item)[:, :])
            if split_out:
                # final pair: per-half outputs so the very last DMA is small
                nc.sync.dma_start(out=jout[jidx]["od"][:, h * 512:(h + 1) * 512],
                                  in_=st[:, sl:sl + 512])
            elif h % 2 == 1:
                nc.sync.dma_start(
                    out=jout[jidx]["od"][:, (h - 1) * 512:(h + 1) * 512],
                    in_=st[:, :])

        prehooks = {
            # between slot 0's gelu and its projection: job0 basis pipeline,
            # first half (pairs 0..8191) so the projections start early
            0: lambda: (emit_recip(0), ladder(0, 128),
                        emit_d1(0, 0, 64), emit_d2(0, 0, 64)),
        }
        hooks = {
            1: lambda: (emit_d1(0, 64, 128), emit_d2(0, 64, 128)),
            2: lambda: [emit_recip(j) for j in range(1, 5)],
            3: lambda: (ladder(128, 384), emit_d1(1), emit_d1(2), emit_d2(1)),
            8: lambda: (ladder(384, NC), emit_d1(4), emit_d1(3), emit_d2(2)),
            10: lambda: emit_d2(4),
            12: lambda: emit_d2(3),
        }
        for k, item in enumerate(SCHED):
            if k >= 3:
                emit_drain(SCHED[k - 3])
            eng = nc.gpsimd if k in POOL_SLOTS else nc.vector
            Ut = upool.tile([128, 2048], dt.float16, tag="u", name="Ut")
            hvt = hvpool.tile([128, 2048], dt.float16, tag="hv", name="hvt")
            emit_ut(*item, Ut=Ut, off=0, eng=eng)
            nc.scalar.activation(hvt[:, :], Ut[:, :], AF.Gelu, bias=0.0)
            if k in prehooks:
                prehooks[k]()
            emit_proj(item, hvt, 0)
            if k in hooks:
                hooks[k]()
        n = len(SCHED)
        for k in range(n, n + 3):
            # trailing drains stay off ACT: the gelu stream IS the makespan
            emit_drain(SCHED[k - 3], on_act=(k == n + 2))

        stack.close()

    nc.compile()
    return nc, {}


def _get_program():
    if "prog" not in _PROGRAM_CACHE:
        _PROGRAM_CACHE["prog"] = _build_program()
    return _PROGRAM_CACHE["prog"]


# ---------------------------------------------------------------------------
# host side


def _prep_core_inputs(core_jobs, pos, tvT_all, n2_all, consts):
    """Build the input map for one core."""
    m = dict(consts)
    for jidx, (b, i0, I, j0, J, mirror) in enumerate(core_jobs):
        p = pos[b]
        n2 = n2_all[b]
        tvT = tvT_all[b]
        fg = np.empty((5, I + J), F32)
        fg[0:3, :I] = -2.0 * p[i0:i0 + I].T
        fg[3, :I] = n2[i0:i0 + I]
        fg[4, :I] = 1.0
        fg[0:3, I:] = p[j0:j0 + J].T
        fg[3, I:] = 1.0
        fg[4, I:] = n2[j0:j0 + J] + 1.0
        tv = np.empty((K, 2 * I + J), F16)
        tv[:, 0:2 * I] = np.repeat(tvT[:, i0:i0 + I], 2, axis=1)
        tv[:, 2 * I:] = tvT[:, j0:j0 + J]
        m[f"fg{jidx}"] = np.ascontiguousarray(fg)
        m[f"tv{jidx}"] = np.ascontiguousarray(tv)
    return m


_RUNNER_CACHE = {}


def _get_runner(nc):
    """Compile (once) a jitted shard_map over the 8 cores with donated,
    device-side-created zero output buffers."""
    if "r" in _RUNNER_CACHE:
        return _RUNNER_CACHE["r"]
    _ensure_concourse()
    import jax
    import jax.numpy as jnp
    from jax.sharding import Mesh, NamedSharding, PartitionSpec
    from jax.experimental.shard_map import shard_map
    from concourse import mybir
    from concourse.bass2jax import (_bass_exec_p, install_neuronx_cc_hook,
                                    partition_id_tensor)

    install_neuronx_cc_hook()

    in_names, out_names, out_avals = [], [], []
    partition_name = (nc.partition_id_tensor.name
                      if nc.partition_id_tensor else None)
    for alloc in nc.m.functions[0].allocations:
        if not isinstance(alloc, mybir.MemoryLocationSet):
            continue
        name = alloc.memorylocations[0].name
        if alloc.kind == "ExternalInput":
            if name != partition_name:
                in_names.append(name)
        elif alloc.kind == "ExternalOutput":
            out_names.append(name)
            out_avals.append(jax.core.ShapedArray(
                tuple(alloc.tensor_shape), mybir.dt.np(alloc.dtype)))
    n_params = len(in_names)
    n_outs = len(out_avals)
    all_in_names = list(in_names) + list(out_names)
    if partition_name is not None:
        all_in_names.append(partition_name)

    def _body(*args):
        operands = list(args)
        if partition_name is not None:
            operands.append(partition_id_tensor())
        outs = _bass_exec_p.bind(
            *operands, out_avals=tuple(out_avals),
            in_names=tuple(all_in_names), out_names=tuple(out_names),
            lowering_input_output_aliases=(), sim_require_finite=True,
            sim_require_nnan=True, nc=nc)
        return tuple(outs)

    devices = jax.devices()[:8]
    mesh = Mesh(np.asarray(devices), ("core",))
    in_specs = (PartitionSpec("core"),) * (n_params + n_outs)
    out_specs = (PartitionSpec("core"),) * n_outs
    donate = tuple(range(n_params, n_params + n_outs))
    sharded = jax.jit(
        shard_map(_body, mesh=mesh, in_specs=in_specs, out_specs=out_specs,
                  check_rep=False),
        donate_argnums=donate, keep_unused=True)

    zshapes = [(8 * a.shape[0], *a.shape[1:]) for a in out_avals]
    zdtypes = [a.dtype for a in out_avals]
    mk = jax.jit(lambda: tuple(jnp.zeros(s, d)
                               for s, d in zip(zshapes, zdtypes)),
                 out_shardings=tuple(
                     NamedSharding(mesh, PartitionSpec("core"))
                     for _ in range(n_outs)))

    _RUNNER_CACHE["r"] = (sharded, mk, in_names, out_names, out_avals)
    return _RUNNER_CACHE["r"]


def _run_on_device(nc, in_maps):
    import jax

    sharded, mk, in_names, out_names, out_avals = _get_runner(nc)
    per_core = [[np.asarray(m[name]) for name in in_names] for m in in_maps]
    concat_in = [np.concatenate([per_core[c][i] for c in range(8)], axis=0)
                 for i in range(len(in_names))]
    out_arrs = jax.block_until_ready(sharded(*concat_in, *mk()))
    results = []
    for c in range(8):
        results.append({
            name: np.asarray(out_arrs[i]).reshape(8, *out_avals[i].shape)[c]
            for i, name in enumerate(out_names)})
    return results


def _decode_direct(arr, I, J):
    """[128, NP/4] fp16 pair-chunked slabs -> [H, I, J] fp32.

    arr[p, chunk*32 + h] holds pair n = chunk*128 + p, n = i*J + j.
    """
    NP = I * J
    nch = NP // 128
    a = arr.astype(F32).reshape(128, nch, 32)     # [p, chunk, h]
    a = a.transpose(1, 0, 2).reshape(NP, 32)      # pair-major [n, h]
    return a.reshape(I, J, 32).transpose(2, 0, 1)


def kernel(**inputs):
    pos = np.ascontiguousarray(np.asarray(inputs["pos"], F32))
    protein_length = int(np.asarray(inputs["protein_length"]))
    means = np.asarray(inputs["means"], np.float64)
    stds = np.asarray(inputs["stds"], np.float64)
    mul_w = np.asarray(inputs["mul_w"], F32)
    bias_w = np.asarray(inputs["bias_w"], F32)
    ow1 = np.asarray(inputs["ow1"], F32)
    ob1 = np.asarray(inputs["ob1"], F32)
    ow2 = np.asarray(inputs["ow2"], F32)
    ob2 = np.asarray(inputs["ob2"], F32)
    vw1 = np.asarray(inputs["vw1"], F32)
    vb1 = np.asarray(inputs["vb1"], F32)
    vw2 = np.asarray(inputs["vw2"], F32)
    vb2 = np.asarray(inputs["vb2"], F32)

    def _fallback():
        return _numpy_reference(pos, np.asarray(inputs["edge_types"]),
                                protein_length, means.astype(F32),
                                np.asarray(stds, F32), mul_w, bias_w, ow1, ob1,
                                ow2, ob2, vw1, vb1, vw2, vb2)

    fast_ok = (
        pos.shape == (B, N, 3)
        and protein_length == PLEN
        and means.shape == (K,)
        and ow1.shape == (K, K) and ow2.shape == (K, H)
        and vw1.shape == (3, K) and vw2.shape == (K, H)
        and np.all(mul_w == mul_w.reshape(-1)[0])
        and np.all(bias_w == bias_w.reshape(-1)[0])
        and np.all(vb1 == 0.0)
        and float(mul_w.reshape(-1)[0]) == 1.0
        and float(bias_w.reshape(-1)[0]) == 0.0
    )
    if not fast_ok:
        return _fallback()

    # host Chebyshev fit of the edge-feature path, with device-exact check
    co, dgrid, fref = _fit_ef_cheb(means, stds,
                                   ow1.astype(np.float64),
                                   ob1.astype(np.float64),
                                   ow2.astype(np.float64))
    if _fit_error(co, dgrid, fref) > 0.012:
        return _fallback()

    consts = {
        "CM": np.ascontiguousarray(co[1:].astype(F16)),             # [D, 32]
        "V2": np.ascontiguousarray(vw2.astype(F16)),
    }
    consts["CM"] = np.concatenate(
        [consts["CM"], np.zeros((K - DCHEB, H), F16)], axis=0)

    n2_all = (pos.astype(np.float64) ** 2).sum(-1).astype(F32)   # [B, N]
    tvT_all = np.stack([(pos[b] @ vw1).T for b in range(B)], 0).astype(F16)
    w3 = (vw1.astype(np.float64) @ vw2.astype(np.float64))       # [3, 32]
    sv_all = np.stack([(pos[b].astype(np.float64) @ w3).T.astype(F32)
                       for b in range(B)], 0)                    # [B, 32, N]
    outb = (ob2 + vb2 + co[0].astype(F32)).astype(F32)           # [32]

    cores = make_jobs()
    in_maps = [_prep_core_inputs(cores[c], pos, tvT_all, n2_all, consts)
               for c in range(8)]

    try:
        nc, meta = _get_program()
        try:
            results = _run_on_device(nc, in_maps)
        except Exception:
            _ensure_concourse()
            from concourse import bass_utils
            res = bass_utils.run_bass_kernel_spmd(nc, in_maps,
                                                  core_ids=list(range(8)))
            results = res.results
    except Exception:
        # No usable device path in this environment: fall back to the exact
        # host implementation so kernel() always returns a correct result.
        return _fallback()

    out = np.zeros((B, H, N, N), F32)
    bias3 = outb[:, None, None]
    for c in range(8):
        for jidx, (b, i0, I, j0, J, mirror) in enumerate(cores[c]):
            od = _decode_direct(results[c][f"od{jidx}"], I, J)
            out[b, :, i0:i0 + I, j0:j0 + J] = od + bias3
            if mirror:
                # mirror tile: gelu(-u) = gelu(u) - u gives
                # om[h,j,i] = od[h,i,j] - sv[h,j] + sv[h,i]
                sv = sv_all[b]
                out[b, :, j0:j0 + J, i0:i0 + I] = (
                    od.transpose(0, 2, 1) + bias3
                    - sv[:, j0:j0 + J, None] + sv[:, None, i0:i0 + I])
    return out


if __name__ == "__main__":
    nc, meta = _get_program()
    print("program built ok")


# revision 64
# speedup vs baseline: 1.0319x; 1.0040x over previous
"""Trainium2 Bass kernel for nn_DistanceBias (gnn_message_passing).

Math (derived from the reference):
  out[b,h,r,c] = ef(dist(r,c))[h] + vec(pos_c - pos_r)[h]   if r < L or c < L
               = 0                                           otherwise
with L = N - protein_length = 256 ligand nodes,
  dist(r,c) = 1/(|pos_r - pos_c|^2 + 1)  in (0, 1],
  ef(d)  = gelu(G(d) @ ow1 + ob1) @ ow2 + ob2,   G_k(d) = gaussian features
  vec(u) = gelu(u @ vw1 + vb1) @ vw2 + vb2.

Key structure exploited:
  * With constant mul_w/bias_w tables the whole edge-feature path ef(d) is a
    smooth scalar function of d in (0,1].  It is fit ONCE on the host as a
    degree-24 Chebyshev series; the device evaluates the basis per pair with a
    cheap fp16 product ladder (V_b = 2*T_b obeys V_{m+n} = V_m*V_n - V_|m-n|)
    and contracts basis x coefficients in the SAME PSUM matmul accumulation as
    the vector-path projection.  This removes the exp activation, the 128x128
    MLP matmuls and their gelu - the activation engine only runs the
    vector-path gelu (one column per pair).
  * dist is symmetric and (vb1 == 0) gelu(-u) = gelu(u) - u, so each unordered
    pair is computed ONCE; mirrors are reconstructed on the host from the
    rank-3 correction sv = pos @ (vw1 @ vw2).
  * The vector-path subtraction u = tvJ[:,j] - tvI[:,i] is built on DVE in the
    2x fp16 mode: the host sends tvI with every column DOUBLED so that all
    three operands end in a packed [1,2] access-pattern dim.
  * The Chebyshev basis is built in pair-tile layout [i, j] and transposed to
    matmul layout [basis, pair] with two large DMAs through a DRAM scratch
    (arbitrary DRAM access patterns make the reshape free).
  * The protein x protein quadrant (56% of output) is exactly zero and never
    touched on device.  Device outputs are fp16 blocked slabs; the host
    reshapes, adds the shared output bias (ob2 + vb2 + c0) and applies
    mirrors.

Work is split into an identical 5-job program per core (3 full 128x128 mirror
blocks + one 64x64 mirror quarter + one 64x128 ordered diag half = 61440
computed pairs per core, exactly 1/8 of the total).
"""

import os
import sys

import numpy as np

# ---------------------------------------------------------------------------
# problem constants (hardcoded per task instructions)
N = 1024
PLEN = 768
LIG = 256
K = 128
H = 32
B = 2
DCHEB = 16  # Chebyshev degree: rows V_1..V_D on device, c_0 folded into bias
A_CONST = (2.0 * 3.14159) ** 0.5  # matches reference PI

F32 = np.float32
F16 = np.float16


def _ensure_concourse():
    try:
        import concourse  # noqa: F401
        return
    except ImportError:
        pass
    for p in ("/opt/trn_rl_repo", "/root/.axon_site/_ro/trn_rl_repo"):
        if os.path.isdir(p) and p not in sys.path:
            sys.path.insert(0, p)
    import concourse  # noqa: F401


# ---------------------------------------------------------------------------
# job tables


def make_jobs():
    """Per-core job lists. Job = (batch, i0, I, j0, J, mirror).

    Fixed per-core structure (same shapes on every core so that a single
    SPMD program serves all 8 cores):
      jobs[0..2] : full 128x128 mirror blocks (LP region)
      jobs[3]    : 64x64 mirror quarter (LL off-diagonal block)
      jobs[4]    : 64x128 ordered diag half (LL diagonal blocks)
    """
    hd = [(0, 64, 0, 128), (64, 64, 0, 128), (128, 64, 128, 128), (192, 64, 128, 128)]
    qq = [(0, 64, 128, 64), (0, 64, 192, 64), (64, 64, 128, 64), (64, 64, 192, 64)]
    lp = []
    for b in range(B):
        for t in range(6):
            for jj in range(2):
                lp.append((b, 256 + 128 * t, 128, 128 * jj, 128, True))
    cores = []
    for c in range(8):
        b = c // 4
        jobs = list(lp[3 * c : 3 * c + 3])
        i0, I, j0, J = qq[c % 4]
        jobs.append((b, i0, I, j0, J, True))
        i0, I, j0, J = hd[c % 4]
        jobs.append((b, i0, I, j0, J, False))
        cores.append(jobs)
    return cores


# job slot shapes shared by the program on every core: (I, J, mirror)
JOB_SLOTS = [(128, 128, True), (128, 128, True), (128, 128, True),
             (64, 64, True), (64, 128, False)]


BASE_DEG = 6  # exact V-recurrence up to here; higher rows are pure products


def _vchain(b):
    """Operands (m, n, k) with V_b = V_m*V_n - V_k, all indices < b."""
    if b % 2 == 0:
        return (b // 2 + 1, b // 2 - 1, 2)
    return ((b + 1) // 2, (b - 1) // 2, 1)


# ---------------------------------------------------------------------------
# numpy fallback (exact reference math) for input shapes/values outside the
# fast path.  kernel.py must be self-contained, so this re-implements the
# reference directly.


def _np_gelu(x):
    z = np.asarray(x, np.float64) / np.sqrt(2.0)
    try:
        from scipy.special import erf
        e = erf(z)
    except ImportError:
        import math
        e = np.vectorize(math.erf)(z)
    return np.asarray(x, np.float64) * (0.5 * (1.0 + e))


def _np_nonlinear(x, w1, b1, w2, b2):
    return (_np_gelu(np.asarray(x, F32) @ w1 + b1) @ w2 + b2).astype(F32)


def _np_gaussian(dist, etype, mul_w, bias_w, means, stds):
    mul = mul_w[etype]
    bias = bias_w[etype]
    x = mul * dist[..., None] + bias
    x = x - means
    std = np.abs(stds) + 1e-5
    return (np.exp(-0.5 * (x / std) ** 2) / (A_CONST * std)).astype(F32)


def _numpy_reference(pos, edge_types, protein_length, means, stds, mul_w, bias_w,
                     ow1, ob1, ow2, ob2, vw1, vb1, vw2, vb2):
    pos = np.asarray(pos, F32)
    Bv, Nv, _ = pos.shape
    P = int(protein_length)
    L = Nv - P
    Hv = ow2.shape[1]
    lig = pos[:, :L]
    prot = pos[:, L:]
    dlm_ll = lig[:, None, :, :] - lig[:, :, None, :]
    dlm_lp = lig[:, None, :, :] - prot[:, :, None, :]
    dist_ll = 1.0 / ((dlm_ll ** 2).sum(-1) + 1.0)
    dist_lp = 1.0 / ((dlm_lp ** 2).sum(-1) + 1.0)
    dlm_ll_h = _np_nonlinear(dlm_ll, vw1, vb1, vw2, vb2)
    dlm_pl_h = _np_nonlinear(-dlm_lp, vw1, vb1, vw2, vb2)
    dlm_lp_h = _np_nonlinear(dlm_lp, vw1, vb1, vw2, vb2)
    g_ll = _np_gaussian(dist_ll, edge_types[:, :L, :L], mul_w, bias_w, means, stds)
    ef_ll = _np_nonlinear(g_ll, ow1, ob1, ow2, ob2)
    g_lp = _np_gaussian(dist_lp, edge_types[:, L:, :L], mul_w, bias_w, means, stds)
    ef_lp = _np_nonlinear(g_lp, ow1, ob1, ow2, ob2)
    ef = np.zeros((Bv, Nv, Nv, Hv), F32)
    ef[:, :L, :L, :] = ef_ll + dlm_ll_h
    ef[:, L:, :L, :] = ef_lp + dlm_lp_h
    ef[:, :L, L:, :] = np.swapaxes(ef_lp + dlm_pl_h, 1, 2)
    return np.transpose(ef, (0, 3, 1, 2)).copy()


# ---------------------------------------------------------------------------
# host-side Chebyshev fit of the edge-feature path


def _vladder_fp16(d32):
    """Simulate the device fp16 basis ladder exactly: V-recurrence up to
    BASE_DEG, then products V_b = V_BASE * V_{b-BASE} for higher degrees."""
    D = DCHEB
    V = [None] * (D + 1)
    V[1] = (4.0 * d32.astype(F32) - 2.0).astype(F16)
    V[2] = ((V[1] * V[1]).astype(F16).astype(F32) - 2.0).astype(F16)
    for b in range(3, BASE_DEG + 1):
        m, n, k = _vchain(b)
        V[b] = ((V[m] * V[n]).astype(F16) - V[k]).astype(F16)
    for b in range(BASE_DEG + 1, D + 1):
        V[b] = (V[BASE_DEG] * V[b - BASE_DEG]).astype(F16)
    return np.stack(V[1:], 0)  # [D, M]


def _fit_ef_cheb(means, stds, ow1, ob1, ow2):
    """Least-squares fit of ef(d) (without ob2) on d in (0,1] against the
    exact fp16 device basis.  Returns co [D+1, 32] (row 0 = constant)."""
    M = 16384
    dgrid = (np.arange(M, dtype=np.float64) + 0.5) / M
    s = np.abs(stds) + 1e-5
    xg = (dgrid[:, None] - means) / s
    G = np.exp(-0.5 * xg * xg) / (A_CONST * s)
    f = _np_gelu(G @ ow1 + ob1) @ ow2       # [M, 32] float64
    Vd = _vladder_fp16(dgrid).astype(np.float64)          # [D, M]
    Bm = np.concatenate([np.ones((1, M)), Vd], axis=0).T  # [M, D+1]
    co, *_ = np.linalg.lstsq(Bm, f, rcond=None)           # [D+1, 32]
    return co, dgrid, f


def _fit_error(co, dgrid, f):
    cm = co[1:].astype(F16).astype(F32)     # [D, 32] as sent to device
    Vd = _vladder_fp16(dgrid)
    est = Vd.astype(F32).T @ cm + co[0][None, :].astype(F32)
    return float(np.abs(est - f).max())


# ---------------------------------------------------------------------------
# device program


_PROGRAM_CACHE = {}


def _build_program():
    """Build the SPMD Bass program (identical for all 8 cores)."""
    _ensure_concourse()
    import contextlib

    import concourse.bass as bass  # noqa: F401
    import concourse.tile as tile
    from concourse import bacc, mybir
    from concourse.tile import add_dep_helper

    dt = mybir.dt
    AF = mybir.ActivationFunctionType
    ALU = mybir.AluOpType

    D = DCHEB
    nc = bacc.Bacc("TRN2", target_bir_lowering=False, debug=False)

    def din(name, shape, dd=None):
        return nc.dram_tensor(name, list(shape), dd or dt.float32,
                              kind="ExternalInput").ap()

    CM = din("CM", (K, H), dt.float16)   # rows 0..D-1 = cheb c_b / 2
    V2 = din("V2", (K, H), dt.float16)   # vw2
    jin, jout, jscr = [], [], []
    for jidx, (I, J, mirror) in enumerate(JOB_SLOTS):
        NP = I * J
        jin.append({
            "fg": din(f"fg{jidx}", (5, I + J)),                    # gl | gr
            "tv": din(f"tv{jidx}", (K, 2 * I + J), dt.float16),    # tvI2 | tvJ
        })
        jout.append({"od": nc.dram_tensor(
            f"od{jidx}", [K, NP // 4], dt.float16, kind="ExternalOutput").ap()})
        jscr.append(nc.dram_tensor(
            f"bs{jidx}", [D, NP], dt.float16, kind="Internal").ap())

    def raw(inst):
        return inst.ins if hasattr(inst, "ins") else inst

    with tile.TileContext(nc) as tc:
        stack = contextlib.ExitStack()
        consts = stack.enter_context(tc.tile_pool(name="consts", bufs=1))
        vpool = stack.enter_context(tc.tile_pool(name="vpool", bufs=1))
        bpool = stack.enter_context(tc.tile_pool(name="bpool", bufs=2))
        bpoolS = stack.enter_context(tc.tile_pool(name="bpoolS", bufs=1))
        upool = stack.enter_context(tc.tile_pool(name="upool", bufs=7))
        hvpool = stack.enter_context(tc.tile_pool(name="hvpool", bufs=8))
        stpool = stack.enter_context(tc.tile_pool(name="stpool", bufs=4))
        psR = stack.enter_context(tc.tile_pool(name="psR", bufs=1, space="PSUM"))
        psO = stack.enter_context(tc.tile_pool(name="psO", bufs=6, space="PSUM"))

        # job0 inputs land first so its pipeline starts immediately
        sbj = [None] * len(JOB_SLOTS)
        for jidx in [0]:
            t = {}
            for kind in ("tv", "fg"):
                shp = list(jin[jidx][kind].shape)
                dd = dt.float16 if kind == "tv" else dt.float32
                t[kind] = consts.tile(shp, dd, name=f"jc_{kind}{jidx}")
                nc.sync.dma_start(out=t[kind][:, :], in_=jin[jidx][kind])
            sbj[jidx] = t
        CM_s = consts.tile([K, H], dt.float16, name="cm")
        nc.sync.dma_start(out=CM_s[:, :], in_=CM)
        V2_s = consts.tile([K, H], dt.float16, name="v2")
        nc.sync.dma_start(out=V2_s[:, :], in_=V2)
        for jidx in range(1, len(JOB_SLOTS)):
            t = {}
            for kind in ("fg", "tv"):
                shp = list(jin[jidx][kind].shape)
                dd = dt.float16 if kind == "tv" else dt.float32
                t[kind] = consts.tile(shp, dd, name=f"jc_{kind}{jidx}")
                nc.sync.dma_start(out=t[kind][:, :], in_=jin[jidx][kind])
            sbj[jidx] = t

        # ---- d = 1/(r^2+1) per job into one [128, 640] tile ---------------
        D_all = vpool.tile([K, 128 * len(JOB_SLOTS)], dt.float32, name="D_all")
        nc.gpsimd.memset(D_all[:, :], 0.0)
        # warmup activation: hoists the auto-inserted Gelu table load (1.3us)
        # into the head idle instead of delaying the first real gelu
        warm = vpool.tile([K, 1], dt.float16, name="warm")
        nc.scalar.activation(warm[:, :], D_all[:, 0:1], AF.Gelu, bias=0.0)

        def emit_recip(jidx):
            I, J, _ = JOB_SLOTS[jidx]
            joff = 128 * jidx
            fg = sbj[jidx]["fg"]
            pR = psR.tile([128, 128], dt.float32, tag="r", name="pR")
            nc.tensor.matmul(pR[:I, :J], fg[:, 0:I], fg[:, I:I + J],
                             start=True, stop=True)
            nc.vector.reciprocal(D_all[:I, joff:joff + J], pR[:I, :J])

        # ---- fp16 basis ladder --------------------------------------------
        # Exact V-recurrence (V_b = 2*T_b) up to BASE_DEG, then pure products
        # V_b = V_BASE * V_{b-BASE}: 29 DVE ops instead of 44, and O(log)
        # dependency depth.  Split in two column ranges: job0's 128 cols
        # first (unblocks the first basis DMA early), then the rest.
        NC = 128 * len(JOB_SLOTS)
        V_all = vpool.tile([K, D * NC], dt.float16, name="V_all")

        def ladder(c0, c1):
            w = c1 - c0

            def V(b):
                return V_all[:, (b - 1) * NC + c0:(b - 1) * NC + c1]

            nc.vector.tensor_scalar(V(1), D_all[:, c0:c1], 4.0, -2.0,
                                    ALU.mult, ALU.add)
            mt = vpool.tile([K, w], dt.float16, name=f"lm0_{c0}",
                            tag=f"lm0_{w}")
            nc.vector.tensor_mul(mt[:, :w], V(1), V(1))
            nc.vector.tensor_scalar(V(2), mt[:, :w], -2.0, None, ALU.add)
            for b in range(3, BASE_DEG + 1):
                m, n, k = _vchain(b)
                mm = vpool.tile([K, w], dt.float16, name=f"lm{b}_{c0}",
                                tag=f"lm{b % 2}_{w}")
                nc.vector.tensor_mul(mm[:, :w], V(m), V(n))
                nc.vector.tensor_sub(V(b), mm[:, :w], V(k))
            for b in range(BASE_DEG + 1, D + 1):
                nc.vector.tensor_mul(V(b), V(BASE_DEG), V(b - BASE_DEG))

        d1_insts = {}

        def emit_d1(jidx, i0=0, i1=None):
            I, J, _ = JOB_SLOTS[jidx]
            if i1 is None:
                i1 = I
            joff = 128 * jidx
            vsl = V_all[:, :].rearrange("p (b c) -> p b c", b=D,
                                        c=NC)[i0:i1, :, joff:joff + J]
            d1_insts[(jidx, i0)] = nc.sync.dma_start(
                out=jscr[jidx][:, i0 * J:i1 * J].rearrange(
                    "b (i j) -> i b j", i=i1 - i0, j=J),
                in_=vsl)

        Bts = {}

        def emit_d2(jidx, i0=0, i1=None):
            I, J, _ = JOB_SLOTS[jidx]
            NP = I * J
            if i1 is None:
                i1 = I
            if jidx in Bts:
                Bt = Bts[jidx]
            elif NP == 16384:
                # jobs 0..2 rotate two big buffers; jobs 3/4 get their own
                # smaller tiles so their loads never wait on buffer reuse
                Bt = bpool.tile([128, NP], dt.float16, tag="B",
                                name=f"Bt{jidx}")
            else:
                Bt = bpoolS.tile([128, NP], dt.float16, tag=f"Bs{jidx}",
                                 name=f"Bt{jidx}")
            d2 = nc.sync.dma_start(out=Bt[0:D, i0 * J:i1 * J],
                                   in_=jscr[jidx][:, i0 * J:i1 * J])
            add_dep_helper(raw(d2), raw(d1_insts[(jidx, i0)]), sync=True,
                           reason="bscratch RAW")
            Bts[jidx] = Bt

        def emit_ut(jidx, h, Ut, off, eng=None, sub=None):
            I, J, _ = JOB_SLOTS[jidx]
            iin = 2048 // J
            iw0 = h * iin
            w = 2048
            if sub is not None:
                iin //= 2
                iw0 += sub * iin
                off += sub * 1024
                w = 1024
            tv = sbj[jidx]["tv"]
            (eng or nc.vector).tensor_tensor(
                Ut[:, off:off + w].rearrange(
                    "p (ii jj j2) -> p ii jj j2", ii=iin, jj=J // 2, j2=2),
                tv[:, 2 * I:2 * I + J][:, None, :].broadcast_to(
                    [128, iin, J]).rearrange(
                    "p ii (jj j2) -> p ii jj j2", j2=2),
                tv[:, 2 * iw0:2 * (iw0 + iin)].rearrange(
                    "p (ii j2) -> p ii j2", j2=2)[:, :, None, :]
                .broadcast_to([128, iin, J // 2, 2]),
                ALU.subtract)

        # ---- schedule -----------------------------------------------------
        # 30 uniform 2048-pair half-group units.  Pool builds a unit's Ut in
        # 4.2us, DVE in 1.1us, ACT consumes one every 1.9us, so Pool covers
        # roughly every other slot while DVE runs the basis ladder pieces.
        # Drains lag their slot by 4 so the in-order DVE queue never parks
        # on an unfinished PSUM tile; output DMAs pair two consecutive
        # halves and follow the odd drain.
        SCHED = []
        for jidx in (0, 1, 2, 4, 3):
            I, J, _ = JOB_SLOTS[jidx]
            SCHED += [(jidx, h) for h in range(I * J // 2048)]
        POOL_SLOTS = {2, 4, 6, 9, 12, 14, 17, 19, 21, 23, 24, 26}

        pOs, sts = {}, {}

        def emit_proj(item, hvt, off):
            jidx, h = item
            Bt = Bts[jidx]
            pO = psO.tile([128, 512], dt.float32, tag="o", name="pO")
            for c in range(16):
                ch0 = h * 2048 + c * 128
                cs = 32 * c
                nc.tensor.matmul(pO[:, cs:cs + 32], Bt[0:D, ch0:ch0 + 128],
                                 CM_s[0:D, :], start=True, stop=False)
                nc.tensor.matmul(pO[:, cs:cs + 32],
                                 hvt[:, off + c * 128:off + (c + 1) * 128],
                                 V2_s[:, :], start=False, stop=True)
            pOs[item] = pO

        def emit_drain(item, on_act=False, split_out=False):
            jidx, h = item
            if h % 2 == 0:
                sts[jidx] = stpool.tile([128, 1024], dt.float16, tag="st",
                                        name="st")
            st = sts[jidx]
            sl = (h % 2) * 512
            if on_act:
                nc.scalar.activation(st[:, sl:sl + 512], pOs.pop(item)[:, :],
                                     AF.Copy)
            else:
                nc.vector.tensor_copy(st[:, sl:sl + 512], pOs.pop(item)[:, :])
            if split_out:
                # final pair: per-half outputs so the very last DMA is small
                nc.sync.dma_start(out=jout[jidx]["od"][:, h * 512:(h + 1) * 512],
                                  in_=st[:, sl:sl + 512])
            elif h % 2 == 1:
                nc.sync.dma_start(
                    out=jout[jidx]["od"][:, (h - 1) * 512:(h + 1) * 512],
                    in_=st[:, :])

        prehooks = {
            # between slot 0's gelu and its projection: job0 basis pipeline,
            # first half (pairs 0..8191) so the projections start early
            0: lambda: (emit_recip(0), ladder(0, 128),
                        emit_d1(0, 0, 64), emit_d2(0, 0, 64)),
        }
        hooks = {
            1: lambda: (emit_d1(0, 64, 128), emit_d2(0, 64, 128)),
            2: lambda: [emit_recip(j) for j in range(1, 5)],
            3: lambda: (ladder(128, 384), emit_d1(1), emit_d1(2), emit_d2(1)),
            8: lambda: (ladder(384, NC), emit_d1(4), emit_d1(3), emit_d2(2)),
            10: lambda: emit_d2(4),
            12: lambda: emit_d2(3),
        }
        for k, item in enumerate(SCHED):
            if k >= 3:
                emit_drain(SCHED[k - 3])
            eng = nc.gpsimd if k in POOL_SLOTS else nc.vector
            Ut = upool.tile([128, 2048], dt.float16, tag="u", name="Ut")
            hvt = hvpool.tile([128, 2048], dt.float16, tag="hv", name="hvt")
            emit_ut(*item, Ut=Ut, off=0, eng=eng)
            nc.scalar.activation(hvt[:, :], Ut[:, :], AF.Gelu, bias=0.0)
            if k in prehooks:
                prehooks[k]()
            emit_proj(item, hvt, 0)
            if k in hooks:
                hooks[k]()
        n = len(SCHED)
        for k in range(n, n + 3):
            # trailing drains stay off ACT: the gelu stream IS the makespan
            emit_drain(SCHED[k - 3], on_act=(k == n + 2))

        stack.close()

    nc.compile()
    return nc, {}


def _get_program():
    if "prog" not in _PROGRAM_CACHE:
        _PROGRAM_CACHE["prog"] = _build_program()
    return _PROGRAM_CACHE["prog"]


# ---------------------------------------------------------------------------
# host side


def _prep_core_inputs(core_jobs, pos, tvT_all, n2_all, consts):
    """Build the input map for one core."""
    m = dict(consts)
    for jidx, (b, i0, I, j0, J, mirror) in enumerate(core_jobs):
        p = pos[b]
        n2 = n2_all[b]
        tvT = tvT_all[b]
        fg = np.empty((5, I + J), F32)
        fg[0:3, :I] = -2.0 * p[i0:i0 + I].T
        fg[3, :I] = n2[i0:i0 + I]
        fg[4, :I] = 1.0
        fg[0:3, I:] = p[j0:j0 + J].T
        fg[3, I:] = 1.0
        fg[4, I:] = n2[j0:j0 + J] + 1.0
        tv = np.empty((K, 2 * I + J), F16)
        tv[:, 0:2 * I] = np.repeat(tvT[:, i0:i0 + I], 2, axis=1)
        tv[:, 2 * I:] = tvT[:, j0:j0 + J]
        m[f"fg{jidx}"] = np.ascontiguousarray(fg)
        m[f"tv{jidx}"] = np.ascontiguousarray(tv)
    return m


_RUNNER_CACHE = {}


def _get_runner(nc):
    """Compile (once) a jitted shard_map over the 8 cores with donated,
    device-side-created zero output buffers."""
    if "r" in _RUNNER_CACHE:
        return _RUNNER_CACHE["r"]
    _ensure_concourse()
    import jax
    import jax.numpy as jnp
    from jax.sharding import Mesh, NamedSharding, PartitionSpec
    from jax.experimental.shard_map import shard_map
    from concourse import mybir
    from concourse.bass2jax import (_bass_exec_p, install_neuronx_cc_hook,
                                    partition_id_tensor)

    install_neuronx_cc_hook()

    in_names, out_names, out_avals = [], [], []
    partition_name = (nc.partition_id_tensor.name
                      if nc.partition_id_tensor else None)
    for alloc in nc.m.functions[0].allocations:
        if not isinstance(alloc, mybir.MemoryLocationSet):
            continue
        name = alloc.memorylocations[0].name
        if alloc.kind == "ExternalInput":
            if name != partition_name:
                in_names.append(name)
        elif alloc.kind == "ExternalOutput":
            out_names.append(name)
            out_avals.append(jax.core.ShapedArray(
                tuple(alloc.tensor_shape), mybir.dt.np(alloc.dtype)))
    n_params = len(in_names)
    n_outs = len(out_avals)
    all_in_names = list(in_names) + list(out_names)
    if partition_name is not None:
        all_in_names.append(partition_name)

    def _body(*args):
        operands = list(args)
        if partition_name is not None:
            operands.append(partition_id_tensor())
        outs = _bass_exec_p.bind(
            *operands, out_avals=tuple(out_avals),
            in_names=tuple(all_in_names), out_names=tuple(out_names),
            lowering_input_output_aliases=(), sim_require_finite=True,
            sim_require_nnan=True, nc=nc)
        return tuple(outs)

    devices = jax.devices()[:8]
    mesh = Mesh(np.asarray(devices), ("core",))
    in_specs = (PartitionSpec("core"),) * (n_params + n_outs)
    out_specs = (PartitionSpec("core"),) * n_outs
    donate = tuple(range(n_params, n_params + n_outs))
    sharded = jax.jit(
        shard_map(_body, mesh=mesh, in_specs=in_specs, out_specs=out_specs,
                  check_rep=False),
        donate_argnums=donate, keep_unused=True)

    zshapes = [(8 * a.shape[0], *a.shape[1:]) for a in out_avals]
    zdtypes = [a.dtype for a in out_avals]
    mk = jax.jit(lambda: tuple(jnp.zeros(s, d)
                               for s, d in zip(zshapes, zdtypes)),
                 out_shardings=tuple(
                     NamedSharding(mesh, PartitionSpec("core"))
                     for _ in range(n_outs)))

    _RUNNER_CACHE["r"] = (sharded, mk, in_names, out_names, out_avals)
    return _RUNNER_CACHE["r"]


def _run_on_device(nc, in_maps):
    import jax

    sharded, mk, in_names, out_names, out_avals = _get_runner(nc)
    per_core = [[np.asarray(m[name]) for name in in_names] for m in in_maps]
    concat_in = [np.concatenate([per_core[c][i] for c in range(8)], axis=0)
                 for i in range(len(in_names))]
    out_arrs = jax.block_until_ready(sharded(*concat_in, *mk()))
    results = []
    for c in range(8):
        results.append({
            name: np.asarray(out_arrs[i]).reshape(8, *out_avals[i].shape)[c]
            for i, name in enumerate(out_names)})
    return results


def _decode_direct(arr, I, J):
    """[128, NP/4] fp16 pair-chunked slabs -> [H, I, J] fp32.

    arr[p, chunk*32 + h] holds pair n = chunk*128 + p, n = i*J + j.
    """
    NP = I * J
    nch = NP // 128
    a = arr.astype(F32).reshape(128, nch, 32)     # [p, chunk, h]
    a = a.transpose(1, 0, 2).reshape(NP, 32)      # pair-major [n, h]
    return a.reshape(I, J, 32).transpose(2, 0, 1)


def kernel(**inputs):
    pos = np.ascontiguousarray(np.asarray(inputs["pos"], F32))
    protein_length = int(np.asarray(inputs["protein_length"]))
    means = np.asarray(inputs["means"], np.float64)
    stds = np.asarray(inputs["stds"], np.float64)
    mul_w = np.asarray(inputs["mul_w"], F32)
    bias_w = np.asarray(inputs["bias_w"], F32)
    ow1 = np.asarray(inputs["ow1"], F32)
    ob1 = np.asarray(inputs["ob1"], F32)
    ow2 = np.asarray(inputs["ow2"], F32)
    ob2 = np.asarray(inputs["ob2"], F32)
    vw1 = np.asarray(inputs["vw1"], F32)
    vb1 = np.asarray(inputs["vb1"], F32)
    vw2 = np.asarray(inputs["vw2"], F32)
    vb2 = np.asarray(inputs["vb2"], F32)

    def _fallback():
        return _numpy_reference(pos, np.asarray(inputs["edge_types"]),
                                protein_length, means.astype(F32),
                                np.asarray(stds, F32), mul_w, bias_w, ow1, ob1,
                                ow2, ob2, vw1, vb1, vw2, vb2)

    fast_ok = (
        pos.shape == (B, N, 3)
        and protein_length == PLEN
        and means.shape == (K,)
        and ow1.shape == (K, K) and ow2.shape == (K, H)
        and vw1.shape == (3, K) and vw2.shape == (K, H)
        and np.all(mul_w == mul_w.reshape(-1)[0])
        and np.all(bias_w == bias_w.reshape(-1)[0])
        and np.all(vb1 == 0.0)
        and float(mul_w.reshape(-1)[0]) == 1.0
        and float(bias_w.reshape(-1)[0]) == 0.0
    )
    if not fast_ok:
        return _fallback()

    # host Chebyshev fit of the edge-feature path, with device-exact check
    co, dgrid, fref = _fit_ef_cheb(means, stds,
                                   ow1.astype(np.float64),
                                   ob1.astype(np.float64),
                                   ow2.astype(np.float64))
    if _fit_error(co, dgrid, fref) > 0.015:
        return _fallback()

    consts = {
        "CM": np.ascontiguousarray(co[1:].astype(F16)),             # [D, 32]
        "V2": np.ascontiguousarray(vw2.astype(F16)),
    }
    consts["CM"] = np.concatenate(
        [consts["CM"], np.zeros((K - DCHEB, H), F16)], axis=0)

    n2_all = (pos.astype(np.float64) ** 2).sum(-1).astype(F32)   # [B, N]
    tvT_all = np.stack([(pos[b] @ vw1).T for b in range(B)], 0).astype(F16)
    w3 = (vw1.astype(np.float64) @ vw2.astype(np.float64))       # [3, 32]
    sv_all = np.stack([(pos[b].astype(np.float64) @ w3).T.astype(F32)
                       for b in range(B)], 0)                    # [B, 32, N]
    outb = (ob2 + vb2 + co[0].astype(F32)).astype(F32)           # [32]

    cores = make_jobs()
    in_maps = [_prep_core_inputs(cores[c], pos, tvT_all, n2_all, consts)
               for c in range(8)]

    try:
        nc, meta = _get_program()
        try:
            results = _run_on_device(nc, in_maps)
        except Exception:
            _ensure_concourse()
            from concourse import bass_utils
            res = bass_utils.run_bass_kernel_spmd(nc, in_maps,
                                                  core_ids=list(range(8)))
            results = res.results
    except Exception:
        # No usable device path in this environment: fall back to the exact
        # host implementation so kernel() always returns a correct result.
        return _fallback()

    out = np.zeros((B, H, N, N), F32)
    bias3 = outb[:, None, None]
    for c in range(8):
        for jidx, (b, i0, I, j0, J, mirror) in enumerate(cores[c]):
            od = _decode_direct(results[c][f"od{jidx}"], I, J)
            out[b, :, i0:i0 + I, j0:j0 + J] = od + bias3
            if mirror:
                # mirror tile: gelu(-u) = gelu(u) - u gives
                # om[h,j,i] = od[h,i,j] - sv[h,j] + sv[h,i]
                sv = sv_all[b]
                out[b, :, j0:j0 + J, i0:i0 + I] = (
                    od.transpose(0, 2, 1) + bias3
                    - sv[:, j0:j0 + J, None] + sv[:, None, i0:i0 + I])
    return out


if __name__ == "__main__":
    nc, meta = _get_program()
    print("program built ok")


# revision 65
# speedup vs baseline: 1.0350x; 1.0030x over previous
"""Trainium2 Bass kernel for nn_DistanceBias (gnn_message_passing).

Math (derived from the reference):
  out[b,h,r,c] = ef(dist(r,c))[h] + vec(pos_c - pos_r)[h]   if r < L or c < L
               = 0                                           otherwise
with L = N - protein_length = 256 ligand nodes,
  dist(r,c) = 1/(|pos_r - pos_c|^2 + 1)  in (0, 1],
  ef(d)  = gelu(G(d) @ ow1 + ob1) @ ow2 + ob2,   G_k(d) = gaussian features
  vec(u) = gelu(u @ vw1 + vb1) @ vw2 + vb2.

Key structure exploited:
  * With constant mul_w/bias_w tables the whole edge-feature path ef(d) is a
    smooth scalar function of d in (0,1].  It is fit ONCE on the host as a
    degree-24 Chebyshev series; the device evaluates the basis per pair with a
    cheap fp16 product ladder (V_b = 2*T_b obeys V_{m+n} = V_m*V_n - V_|m-n|)
    and contracts basis x coefficients in the SAME PSUM matmul accumulation as
    the vector-path projection.  This removes the exp activation, the 128x128
    MLP matmuls and their gelu - the activation engine only runs the
    vector-path gelu (one column per pair).
  * dist is symmetric and (vb1 == 0) gelu(-u) = gelu(u) - u, so each unordered
    pair is computed ONCE; mirrors are reconstructed on the host from the
    rank-3 correction sv = pos @ (vw1 @ vw2).
  * The vector-path subtraction u = tvJ[:,j] - tvI[:,i] is built on DVE in the
    2x fp16 mode: the host sends tvI with every column DOUBLED so that all
    three operands end in a packed [1,2] access-pattern dim.
  * The Chebyshev basis is built in pair-tile layout [i, j] and transposed to
    matmul layout [basis, pair] with two large DMAs through a DRAM scratch
    (arbitrary DRAM access patterns make the reshape free).
  * The protein x protein quadrant (56% of output) is exactly zero and never
    touched on device.  Device outputs are fp16 blocked slabs; the host
    reshapes, adds the shared output bias (ob2 + vb2 + c0) and applies
    mirrors.

Work is split into an identical 5-job program per core (3 full 128x128 mirror
blocks + one 64x64 mirror quarter + one 64x128 ordered diag half = 61440
computed pairs per core, exactly 1/8 of the total).
"""

import os
import sys

import numpy as np

# ---------------------------------------------------------------------------
# problem constants (hardcoded per task instructions)
N = 1024
PLEN = 768
LIG = 256
K = 128
H = 32
B = 2
DCHEB = 15  # Chebyshev degree: rows V_1..V_D on device, c_0 folded into bias
A_CONST = (2.0 * 3.14159) ** 0.5  # matches reference PI

F32 = np.float32
F16 = np.float16


def _ensure_concourse():
    try:
        import concourse  # noqa: F401
        return
    except ImportError:
        pass
    for p in ("/opt/trn_rl_repo", "/root/.axon_site/_ro/trn_rl_repo"):
        if os.path.isdir(p) and p not in sys.path:
            sys.path.insert(0, p)
    import concourse  # noqa: F401


# ---------------------------------------------------------------------------
# job tables


def make_jobs():
    """Per-core job lists. Job = (batch, i0, I, j0, J, mirror).

    Fixed per-core structure (same shapes on every core so that a single
    SPMD program serves all 8 cores):
      jobs[0..2] : full 128x128 mirror blocks (LP region)
      jobs[3]    : 64x64 mirror quarter (LL off-diagonal block)
      jobs[4]    : 64x128 ordered diag half (LL diagonal blocks)
    """
    hd = [(0, 64, 0, 128), (64, 64, 0, 128), (128, 64, 128, 128), (192, 64, 128, 128)]
    qq = [(0, 64, 128, 64), (0, 64, 192, 64), (64, 64, 128, 64), (64, 64, 192, 64)]
    lp = []
    for b in range(B):
        for t in range(6):
            for jj in range(2):
                lp.append((b, 256 + 128 * t, 128, 128 * jj, 128, True))
    cores = []
    for c in range(8):
        b = c // 4
        jobs = list(lp[3 * c : 3 * c + 3])
        i0, I, j0, J = qq[c % 4]
        jobs.append((b, i0, I, j0, J, True))
        i0, I, j0, J = hd[c % 4]
        jobs.append((b, i0, I, j0, J, False))
        cores.append(jobs)
    return cores


# job slot shapes shared by the program on every core: (I, J, mirror)
JOB_SLOTS = [(128, 128, True), (128, 128, True), (128, 128, True),
             (64, 64, True), (64, 128, False)]


BASE_DEG = 6  # exact V-recurrence up to here; higher rows are pure products


def _vchain(b):
    """Operands (m, n, k) with V_b = V_m*V_n - V_k, all indices < b."""
    if b % 2 == 0:
        return (b // 2 + 1, b // 2 - 1, 2)
    return ((b + 1) // 2, (b - 1) // 2, 1)


# ---------------------------------------------------------------------------
# numpy fallback (exact reference math) for input shapes/values outside the
# fast path.  kernel.py must be self-contained, so this re-implements the
# reference directly.


def _np_gelu(x):
    z = np.asarray(x, np.float64) / np.sqrt(2.0)
    try:
        from scipy.special import erf
        e = erf(z)
    except ImportError:
        import math
        e = np.vectorize(math.erf)(z)
    return np.asarray(x, np.float64) * (0.5 * (1.0 + e))


def _np_nonlinear(x, w1, b1, w2, b2):
    return (_np_gelu(np.asarray(x, F32) @ w1 + b1) @ w2 + b2).astype(F32)


def _np_gaussian(dist, etype, mul_w, bias_w, means, stds):
    mul = mul_w[etype]
    bias = bias_w[etype]
    x = mul * dist[..., None] + bias
    x = x - means
    std = np.abs(stds) + 1e-5
    return (np.exp(-0.5 * (x / std) ** 2) / (A_CONST * std)).astype(F32)


def _numpy_reference(pos, edge_types, protein_length, means, stds, mul_w, bias_w,
                     ow1, ob1, ow2, ob2, vw1, vb1, vw2, vb2):
    pos = np.asarray(pos, F32)
    Bv, Nv, _ = pos.shape
    P = int(protein_length)
    L = Nv - P
    Hv = ow2.shape[1]
    lig = pos[:, :L]
    prot = pos[:, L:]
    dlm_ll = lig[:, None, :, :] - lig[:, :, None, :]
    dlm_lp = lig[:, None, :, :] - prot[:, :, None, :]
    dist_ll = 1.0 / ((dlm_ll ** 2).sum(-1) + 1.0)
    dist_lp = 1.0 / ((dlm_lp ** 2).sum(-1) + 1.0)
    dlm_ll_h = _np_nonlinear(dlm_ll, vw1, vb1, vw2, vb2)
    dlm_pl_h = _np_nonlinear(-dlm_lp, vw1, vb1, vw2, vb2)
    dlm_lp_h = _np_nonlinear(dlm_lp, vw1, vb1, vw2, vb2)
    g_ll = _np_gaussian(dist_ll, edge_types[:, :L, :L], mul_w, bias_w, means, stds)
    ef_ll = _np_nonlinear(g_ll, ow1, ob1, ow2, ob2)
    g_lp = _np_gaussian(dist_lp, edge_types[:, L:, :L], mul_w, bias_w, means, stds)
    ef_lp = _np_nonlinear(g_lp, ow1, ob1, ow2, ob2)
    ef = np.zeros((Bv, Nv, Nv, Hv), F32)
    ef[:, :L, :L, :] = ef_ll + dlm_ll_h
    ef[:, L:, :L, :] = ef_lp + dlm_lp_h
    ef[:, :L, L:, :] = np.swapaxes(ef_lp + dlm_pl_h, 1, 2)
    return np.transpose(ef, (0, 3, 1, 2)).copy()


# ---------------------------------------------------------------------------
# host-side Chebyshev fit of the edge-feature path


def _vladder_fp16(d32):
    """Simulate the device fp16 basis ladder exactly: V-recurrence up to
    BASE_DEG, then products V_b = V_BASE * V_{b-BASE} for higher degrees."""
    D = DCHEB
    V = [None] * (D + 1)
    V[1] = (4.0 * d32.astype(F32) - 2.0).astype(F16)
    V[2] = ((V[1] * V[1]).astype(F16).astype(F32) - 2.0).astype(F16)
    for b in range(3, BASE_DEG + 1):
        m, n, k = _vchain(b)
        V[b] = ((V[m] * V[n]).astype(F16) - V[k]).astype(F16)
    for b in range(BASE_DEG + 1, D + 1):
        V[b] = (V[BASE_DEG] * V[b - BASE_DEG]).astype(F16)
    return np.stack(V[1:], 0)  # [D, M]


def _fit_ef_cheb(means, stds, ow1, ob1, ow2):
    """Least-squares fit of ef(d) (without ob2) on d in (0,1] against the
    exact fp16 device basis.  Returns co [D+1, 32] (row 0 = constant)."""
    M = 16384
    dgrid = (np.arange(M, dtype=np.float64) + 0.5) / M
    s = np.abs(stds) + 1e-5
    xg = (dgrid[:, None] - means) / s
    G = np.exp(-0.5 * xg * xg) / (A_CONST * s)
    f = _np_gelu(G @ ow1 + ob1) @ ow2       # [M, 32] float64
    Vd = _vladder_fp16(dgrid).astype(np.float64)          # [D, M]
    Bm = np.concatenate([np.ones((1, M)), Vd], axis=0).T  # [M, D+1]
    co, *_ = np.linalg.lstsq(Bm, f, rcond=None)           # [D+1, 32]
    return co, dgrid, f


def _fit_error(co, dgrid, f):
    cm = co[1:].astype(F16).astype(F32)     # [D, 32] as sent to device
    Vd = _vladder_fp16(dgrid)
    est = Vd.astype(F32).T @ cm + co[0][None, :].astype(F32)
    return float(np.abs(est - f).max())


# ---------------------------------------------------------------------------
# device program


_PROGRAM_CACHE = {}


def _build_program():
    """Build the SPMD Bass program (identical for all 8 cores)."""
    _ensure_concourse()
    import contextlib

    import concourse.bass as bass  # noqa: F401
    import concourse.tile as tile
    from concourse import bacc, mybir
    from concourse.tile import add_dep_helper

    dt = mybir.dt
    AF = mybir.ActivationFunctionType
    ALU = mybir.AluOpType

    D = DCHEB
    nc = bacc.Bacc("TRN2", target_bir_lowering=False, debug=False)

    def din(name, shape, dd=None):
        return nc.dram_tensor(name, list(shape), dd or dt.float32,
                              kind="ExternalInput").ap()

    CM = din("CM", (K, H), dt.float16)   # rows 0..D-1 = cheb c_b / 2
    V2 = din("V2", (K, H), dt.float16)   # vw2
    jin, jout, jscr = [], [], []
    for jidx, (I, J, mirror) in enumerate(JOB_SLOTS):
        NP = I * J
        jin.append({
            "fg": din(f"fg{jidx}", (5, I + J)),                    # gl | gr
            "tv": din(f"tv{jidx}", (K, 2 * I + J), dt.float16),    # tvI2 | tvJ
        })
        jout.append({"od": nc.dram_tensor(
            f"od{jidx}", [K, NP // 4], dt.float16, kind="ExternalOutput").ap()})
        jscr.append(nc.dram_tensor(
            f"bs{jidx}", [D, NP], dt.float16, kind="Internal").ap())

    def raw(inst):
        return inst.ins if hasattr(inst, "ins") else inst

    with tile.TileContext(nc) as tc:
        stack = contextlib.ExitStack()
        consts = stack.enter_context(tc.tile_pool(name="consts", bufs=1))
        vpool = stack.enter_context(tc.tile_pool(name="vpool", bufs=1))
        bpool = stack.enter_context(tc.tile_pool(name="bpool", bufs=2))
        bpoolS = stack.enter_context(tc.tile_pool(name="bpoolS", bufs=1))
        upool = stack.enter_context(tc.tile_pool(name="upool", bufs=7))
        hvpool = stack.enter_context(tc.tile_pool(name="hvpool", bufs=8))
        stpool = stack.enter_context(tc.tile_pool(name="stpool", bufs=4))
        psR = stack.enter_context(tc.tile_pool(name="psR", bufs=1, space="PSUM"))
        psO = stack.enter_context(tc.tile_pool(name="psO", bufs=6, space="PSUM"))

        # job0 inputs land first so its pipeline starts immediately
        sbj = [None] * len(JOB_SLOTS)
        for jidx in [0]:
            t = {}
            for kind in ("tv", "fg"):
                shp = list(jin[jidx][kind].shape)
                dd = dt.float16 if kind == "tv" else dt.float32
                t[kind] = consts.tile(shp, dd, name=f"jc_{kind}{jidx}")
                nc.sync.dma_start(out=t[kind][:, :], in_=jin[jidx][kind])
            sbj[jidx] = t
        CM_s = consts.tile([K, H], dt.float16, name="cm")
        nc.sync.dma_start(out=CM_s[:, :], in_=CM)
        V2_s = consts.tile([K, H], dt.float16, name="v2")
        nc.sync.dma_start(out=V2_s[:, :], in_=V2)
        for jidx in range(1, len(JOB_SLOTS)):
            t = {}
            for kind in ("fg", "tv"):
                shp = list(jin[jidx][kind].shape)
                dd = dt.float16 if kind == "tv" else dt.float32
                t[kind] = consts.tile(shp, dd, name=f"jc_{kind}{jidx}")
                nc.sync.dma_start(out=t[kind][:, :], in_=jin[jidx][kind])
            sbj[jidx] = t

        # ---- d = 1/(r^2+1) per job into one [128, 640] tile ---------------
        D_all = vpool.tile([K, 128 * len(JOB_SLOTS)], dt.float32, name="D_all")
        nc.gpsimd.memset(D_all[:, :], 0.0)
        # warmup activation: hoists the auto-inserted Gelu table load (1.3us)
        # into the head idle instead of delaying the first real gelu
        warm = vpool.tile([K, 1], dt.float16, name="warm")
        nc.scalar.activation(warm[:, :], D_all[:, 0:1], AF.Gelu, bias=0.0)

        def emit_recip(jidx):
            I, J, _ = JOB_SLOTS[jidx]
            joff = 128 * jidx
            fg = sbj[jidx]["fg"]
            pR = psR.tile([128, 128], dt.float32, tag="r", name="pR")
            nc.tensor.matmul(pR[:I, :J], fg[:, 0:I], fg[:, I:I + J],
                             start=True, stop=True)
            nc.vector.reciprocal(D_all[:I, joff:joff + J], pR[:I, :J])

        # ---- fp16 basis ladder --------------------------------------------
        # Exact V-recurrence (V_b = 2*T_b) up to BASE_DEG, then pure products
        # V_b = V_BASE * V_{b-BASE}: 29 DVE ops instead of 44, and O(log)
        # dependency depth.  Split in two column ranges: job0's 128 cols
        # first (unblocks the first basis DMA early), then the rest.
        NC = 128 * len(JOB_SLOTS)
        V_all = vpool.tile([K, D * NC], dt.float16, name="V_all")

        def ladder(c0, c1):
            w = c1 - c0

            def V(b):
                return V_all[:, (b - 1) * NC + c0:(b - 1) * NC + c1]

            nc.vector.tensor_scalar(V(1), D_all[:, c0:c1], 4.0, -2.0,
                                    ALU.mult, ALU.add)
            mt = vpool.tile([K, w], dt.float16, name=f"lm0_{c0}",
                            tag=f"lm0_{w}")
            nc.vector.tensor_mul(mt[:, :w], V(1), V(1))
            nc.vector.tensor_scalar(V(2), mt[:, :w], -2.0, None, ALU.add)
            for b in range(3, BASE_DEG + 1):
                m, n, k = _vchain(b)
                mm = vpool.tile([K, w], dt.float16, name=f"lm{b}_{c0}",
                                tag=f"lm{b % 2}_{w}")
                nc.vector.tensor_mul(mm[:, :w], V(m), V(n))
                nc.vector.tensor_sub(V(b), mm[:, :w], V(k))
            for b in range(BASE_DEG + 1, D + 1):
                nc.vector.tensor_mul(V(b), V(BASE_DEG), V(b - BASE_DEG))

        d1_insts = {}

        def emit_d1(jidx, i0=0, i1=None):
            I, J, _ = JOB_SLOTS[jidx]
            if i1 is None:
                i1 = I
            joff = 128 * jidx
            vsl = V_all[:, :].rearrange("p (b c) -> p b c", b=D,
                                        c=NC)[i0:i1, :, joff:joff + J]
            d1_insts[(jidx, i0)] = nc.sync.dma_start(
                out=jscr[jidx][:, i0 * J:i1 * J].rearrange(
                    "b (i j) -> i b j", i=i1 - i0, j=J),
                in_=vsl)

        Bts = {}

        def emit_d2(jidx, i0=0, i1=None):
            I, J, _ = JOB_SLOTS[jidx]
            NP = I * J
            if i1 is None:
                i1 = I
            if jidx in Bts:
                Bt = Bts[jidx]
            elif NP == 16384:
                # jobs 0..2 rotate two big buffers; jobs 3/4 get their own
                # smaller tiles so their loads never wait on buffer reuse
                Bt = bpool.tile([128, NP], dt.float16, tag="B",
                                name=f"Bt{jidx}")
            else:
                Bt = bpoolS.tile([128, NP], dt.float16, tag=f"Bs{jidx}",
                                 name=f"Bt{jidx}")
            d2 = nc.sync.dma_start(out=Bt[0:D, i0 * J:i1 * J],
                                   in_=jscr[jidx][:, i0 * J:i1 * J])
            add_dep_helper(raw(d2), raw(d1_insts[(jidx, i0)]), sync=True,
                           reason="bscratch RAW")
            Bts[jidx] = Bt

        def emit_ut(jidx, h, Ut, off, eng=None, sub=None):
            I, J, _ = JOB_SLOTS[jidx]
            iin = 2048 // J
            iw0 = h * iin
            w = 2048
            if sub is not None:
                iin //= 2
                iw0 += sub * iin
                off += sub * 1024
                w = 1024
            tv = sbj[jidx]["tv"]
            (eng or nc.vector).tensor_tensor(
                Ut[:, off:off + w].rearrange(
                    "p (ii jj j2) -> p ii jj j2", ii=iin, jj=J // 2, j2=2),
                tv[:, 2 * I:2 * I + J][:, None, :].broadcast_to(
                    [128, iin, J]).rearrange(
                    "p ii (jj j2) -> p ii jj j2", j2=2),
                tv[:, 2 * iw0:2 * (iw0 + iin)].rearrange(
                    "p (ii j2) -> p ii j2", j2=2)[:, :, None, :]
                .broadcast_to([128, iin, J // 2, 2]),
                ALU.subtract)

        # ---- schedule -----------------------------------------------------
        # 30 uniform 2048-pair half-group units.  Pool builds a unit's Ut in
        # 4.2us, DVE in 1.1us, ACT consumes one every 1.9us, so Pool covers
        # roughly every other slot while DVE runs the basis ladder pieces.
        # Drains lag their slot by 4 so the in-order DVE queue never parks
        # on an unfinished PSUM tile; output DMAs pair two consecutive
        # halves and follow the odd drain.
        SCHED = []
        for jidx in (0, 1, 2, 4, 3):
            I, J, _ = JOB_SLOTS[jidx]
            SCHED += [(jidx, h) for h in range(I * J // 2048)]
        POOL_SLOTS = {2, 4, 6, 9, 12, 14, 17, 19, 21, 23, 24, 26}

        pOs, sts = {}, {}

        def emit_proj(item, hvt, off):
            jidx, h = item
            Bt = Bts[jidx]
            pO = psO.tile([128, 512], dt.float32, tag="o", name="pO")
            for c in range(16):
                ch0 = h * 2048 + c * 128
                cs = 32 * c
                nc.tensor.matmul(pO[:, cs:cs + 32], Bt[0:D, ch0:ch0 + 128],
                                 CM_s[0:D, :], start=True, stop=False)
                nc.tensor.matmul(pO[:, cs:cs + 32],
                                 hvt[:, off + c * 128:off + (c + 1) * 128],
                                 V2_s[:, :], start=False, stop=True)
            pOs[item] = pO

        def emit_drain(item, on_act=False, split_out=False):
            jidx, h = item
            if h % 2 == 0:
                sts[jidx] = stpool.tile([128, 1024], dt.float16, tag="st",
                                        name="st")
            st = sts[jidx]
            sl = (h % 2) * 512
            if on_act:
                nc.scalar.activation(st[:, sl:sl + 512], pOs.pop(item)[:, :],
                                     AF.Copy)
            else:
                nc.vector.tensor_copy(st[:, sl:sl + 512], pOs.pop(item)[:, :])
            if split_out:
                # final pair: per-half outputs so the very last DMA is small
                nc.sync.dma_start(out=jout[jidx]["od"][:, h * 512:(h + 1) * 512],
                                  in_=st[:, sl:sl + 512])
            elif h % 2 == 1:
                nc.sync.dma_start(
                    out=jout[jidx]["od"][:, (h - 1) * 512:(h + 1) * 512],
                    in_=st[:, :])

        prehooks = {
            # between slot 0's gelu and its projection: job0 basis pipeline,
            # first half (pairs 0..8191) so the projections start early
            0: lambda: (emit_recip(0), ladder(0, 128),
                        emit_d1(0, 0, 64), emit_d2(0, 0, 64)),
        }
        hooks = {
            1: lambda: (emit_d1(0, 64, 128), emit_d2(0, 64, 128)),
            2: lambda: [emit_recip(j) for j in range(1, 5)],
            3: lambda: (ladder(128, 384), emit_d1(1), emit_d1(2), emit_d2(1)),
            8: lambda: (ladder(384, NC), emit_d1(4), emit_d1(3), emit_d2(2)),
            10: lambda: emit_d2(4),
            12: lambda: emit_d2(3),
        }
        for k, item in enumerate(SCHED):
            if k >= 3:
                emit_drain(SCHED[k - 3])
            eng = nc.gpsimd if k in POOL_SLOTS else nc.vector
            Ut = upool.tile([128, 2048], dt.float16, tag="u", name="Ut")
            hvt = hvpool.tile([128, 2048], dt.float16, tag="hv", name="hvt")
            emit_ut(*item, Ut=Ut, off=0, eng=eng)
            nc.scalar.activation(hvt[:, :], Ut[:, :], AF.Gelu, bias=0.0)
            if k in prehooks:
                prehooks[k]()
            emit_proj(item, hvt, 0)
            if k in hooks:
                hooks[k]()
        n = len(SCHED)
        for k in range(n, n + 3):
            # trailing drains stay off ACT: the gelu stream IS the makespan
            emit_drain(SCHED[k - 3], on_act=(k == n + 2))

        stack.close()

    nc.compile()
    return nc, {}


def _get_program():
    if "prog" not in _PROGRAM_CACHE:
        _PROGRAM_CACHE["prog"] = _build_program()
    return _PROGRAM_CACHE["prog"]


# ---------------------------------------------------------------------------
# host side


def _prep_core_inputs(core_jobs, pos, tvT_all, n2_all, consts):
    """Build the input map for one core."""
    m = dict(consts)
    for jidx, (b, i0, I, j0, J, mirror) in enumerate(core_jobs):
        p = pos[b]
        n2 = n2_all[b]
        tvT = tvT_all[b]
        fg = np.empty((5, I + J), F32)
        fg[0:3, :I] = -2.0 * p[i0:i0 + I].T
        fg[3, :I] = n2[i0:i0 + I]
        fg[4, :I] = 1.0
        fg[0:3, I:] = p[j0:j0 + J].T
        fg[3, I:] = 1.0
        fg[4, I:] = n2[j0:j0 + J] + 1.0
        tv = np.empty((K, 2 * I + J), F16)
        tv[:, 0:2 * I] = np.repeat(tvT[:, i0:i0 + I], 2, axis=1)
        tv[:, 2 * I:] = tvT[:, j0:j0 + J]
        m[f"fg{jidx}"] = np.ascontiguousarray(fg)
        m[f"tv{jidx}"] = np.ascontiguousarray(tv)
    return m


_RUNNER_CACHE = {}


def _get_runner(nc):
    """Compile (once) a jitted shard_map over the 8 cores with donated,
    device-side-created zero output buffers."""
    if "r" in _RUNNER_CACHE:
        return _RUNNER_CACHE["r"]
    _ensure_concourse()
    import jax
    import jax.numpy as jnp
    from jax.sharding import Mesh, NamedSharding, PartitionSpec
    from jax.experimental.shard_map import shard_map
    from concourse import mybir
    from concourse.bass2jax import (_bass_exec_p, install_neuronx_cc_hook,
                                    partition_id_tensor)

    install_neuronx_cc_hook()

    in_names, out_names, out_avals = [], [], []
    partition_name = (nc.partition_id_tensor.name
                      if nc.partition_id_tensor else None)
    for alloc in nc.m.functions[0].allocations:
        if not isinstance(alloc, mybir.MemoryLocationSet):
            continue
        name = alloc.memorylocations[0].name
        if alloc.kind == "ExternalInput":
            if name != partition_name:
                in_names.append(name)
        elif alloc.kind == "ExternalOutput":
            out_names.append(name)
            out_avals.append(jax.core.ShapedArray(
                tuple(alloc.tensor_shape), mybir.dt.np(alloc.dtype)))
    n_params = len(in_names)
    n_outs = len(out_avals)
    all_in_names = list(in_names) + list(out_names)
    if partition_name is not None:
        all_in_names.append(partition_name)

    def _body(*args):
        operands = list(args)
        if partition_name is not None:
            operands.append(partition_id_tensor())
        outs = _bass_exec_p.bind(
            *operands, out_avals=tuple(out_avals),
            in_names=tuple(all_in_names), out_names=tuple(out_names),
            lowering_input_output_aliases=(), sim_require_finite=True,
            sim_require_nnan=True, nc=nc)
        return tuple(outs)

    devices = jax.devices()[:8]
    mesh = Mesh(np.asarray(devices), ("core",))
    in_specs = (PartitionSpec("core"),) * (n_params + n_outs)
    out_specs = (PartitionSpec("core"),) * n_outs
    donate = tuple(range(n_params, n_params + n_outs))
    sharded = jax.jit(
        shard_map(_body, mesh=mesh, in_specs=in_specs, out_specs=out_specs,
                  check_rep=False),
        donate_argnums=donate, keep_unused=True)

    zshapes = [(8 * a.shape[0], *a.shape[1:]) for a in out_avals]
    zdtypes = [a.dtype for a in out_avals]
    mk = jax.jit(lambda: tuple(jnp.zeros(s, d)
                               for s, d in zip(zshapes, zdtypes)),
                 out_shardings=tuple(
                     NamedSharding(mesh, PartitionSpec("core"))
                     for _ in range(n_outs)))

    _RUNNER_CACHE["r"] = (sharded, mk, in_names, out_names, out_avals)
    return _RUNNER_CACHE["r"]


def _run_on_device(nc, in_maps):
    import jax

    sharded, mk, in_names, out_names, out_avals = _get_runner(nc)
    per_core = [[np.asarray(m[name]) for name in in_names] for m in in_maps]
    concat_in = [np.concatenate([per_core[c][i] for c in range(8)], axis=0)
                 for i in range(len(in_names))]
    out_arrs = jax.block_until_ready(sharded(*concat_in, *mk()))
    results = []
    for c in range(8):
        results.append({
            name: np.asarray(out_arrs[i]).reshape(8, *out_avals[i].shape)[c]
            for i, name in enumerate(out_names)})
    return results


def _decode_direct(arr, I, J):
    """[128, NP/4] fp16 pair-chunked slabs -> [H, I, J] fp32.

    arr[p, chunk*32 + h] holds pair n = chunk*128 + p, n = i*J + j.
    """
    NP = I * J
    nch = NP // 128
    a = arr.astype(F32).reshape(128, nch, 32)     # [p, chunk, h]
    a = a.transpose(1, 0, 2).reshape(NP, 32)      # pair-major [n, h]
    return a.reshape(I, J, 32).transpose(2, 0, 1)


def kernel(**inputs):
    pos = np.ascontiguousarray(np.asarray(inputs["pos"], F32))
    protein_length = int(np.asarray(inputs["protein_length"]))
    means = np.asarray(inputs["means"], np.float64)
    stds = np.asarray(inputs["stds"], np.float64)
    mul_w = np.asarray(inputs["mul_w"], F32)
    bias_w = np.asarray(inputs["bias_w"], F32)
    ow1 = np.asarray(inputs["ow1"], F32)
    ob1 = np.asarray(inputs["ob1"], F32)
    ow2 = np.asarray(inputs["ow2"], F32)
    ob2 = np.asarray(inputs["ob2"], F32)
    vw1 = np.asarray(inputs["vw1"], F32)
    vb1 = np.asarray(inputs["vb1"], F32)
    vw2 = np.asarray(inputs["vw2"], F32)
    vb2 = np.asarray(inputs["vb2"], F32)

    def _fallback():
        return _numpy_reference(pos, np.asarray(inputs["edge_types"]),
                                protein_length, means.astype(F32),
                                np.asarray(stds, F32), mul_w, bias_w, ow1, ob1,
                                ow2, ob2, vw1, vb1, vw2, vb2)

    fast_ok = (
        pos.shape == (B, N, 3)
        and protein_length == PLEN
        and means.shape == (K,)
        and ow1.shape == (K, K) and ow2.shape == (K, H)
        and vw1.shape == (3, K) and vw2.shape == (K, H)
        and np.all(mul_w == mul_w.reshape(-1)[0])
        and np.all(bias_w == bias_w.reshape(-1)[0])
        and np.all(vb1 == 0.0)
        and float(mul_w.reshape(-1)[0]) == 1.0
        and float(bias_w.reshape(-1)[0]) == 0.0
    )
    if not fast_ok:
        return _fallback()

    # host Chebyshev fit of the edge-feature path, with device-exact check
    co, dgrid, fref = _fit_ef_cheb(means, stds,
                                   ow1.astype(np.float64),
                                   ob1.astype(np.float64),
                                   ow2.astype(np.float64))
    if _fit_error(co, dgrid, fref) > 0.015:
        return _fallback()

    consts = {
        "CM": np.ascontiguousarray(co[1:].astype(F16)),             # [D, 32]
        "V2": np.ascontiguousarray(vw2.astype(F16)),
    }
    consts["CM"] = np.concatenate(
        [consts["CM"], np.zeros((K - DCHEB, H), F16)], axis=0)

    n2_all = (pos.astype(np.float64) ** 2).sum(-1).astype(F32)   # [B, N]
    tvT_all = np.stack([(pos[b] @ vw1).T for b in range(B)], 0).astype(F16)
    w3 = (vw1.astype(np.float64) @ vw2.astype(np.float64))       # [3, 32]
    sv_all = np.stack([(pos[b].astype(np.float64) @ w3).T.astype(F32)
                       for b in range(B)], 0)                    # [B, 32, N]
    outb = (ob2 + vb2 + co[0].astype(F32)).astype(F32)           # [32]

    cores = make_jobs()
    in_maps = [_prep_core_inputs(cores[c], pos, tvT_all, n2_all, consts)
               for c in range(8)]

    try:
        nc, meta = _get_program()
        try:
            results = _run_on_device(nc, in_maps)
        except Exception:
            _ensure_concourse()
            from concourse import bass_utils
            res = bass_utils.run_bass_kernel_spmd(nc, in_maps,
                                                  core_ids=list(range(8)))
            results = res.results
    except Exception:
        # No usable device path in this environment: fall back to the exact
        # host implementation so kernel() always returns a correct result.
        return _fallback()

    out = np.zeros((B, H, N, N), F32)
    bias3 = outb[:, None, None]
    for c in range(8):
        for jidx, (b, i0, I, j0, J, mirror) in enumerate(cores[c]):
            od = _decode_direct(results[c][f"od{jidx}"], I, J)
            out[b, :, i0:i0 + I, j0:j0 + J] = od + bias3
            if mirror:
                # mirror tile: gelu(-u) = gelu(u) - u gives
                # om[h,j,i] = od[h,i,j] - sv[h,j] + sv[h,i]
                sv = sv_all[b]
                out[b, :, j0:j0 + J, i0:i0 + I] = (
                    od.transpose(0, 2, 1) + bias3
                    - sv[:, j0:j0 + J, None] + sv[:, None, i0:i0 + I])
    return out


if __name__ == "__main__":
    nc, meta = _get_program()
    print("program built ok")
